# revision 23
# baseline (speedup 1.0000x reference)
"""Trainium2 Bass kernel for nn_BaselineModel_80796924772520 (dense_cnn).

Self-contained: kernel(**inputs) -> np.ndarray [512, 7] float32.

Strategy: pure data parallelism over 8 NeuronCores (64 images each).
 - BN folded into conv weights/biases on host (eval-mode BN is affine).
 - fc1/fc2/att collapse into one linear map W_eff [64, 2304] on host
   (reference has no nonlinearity between them).
 - conv1 (C_in=1, K=9): x-parity decomposition on two PE row-quadrants.
   Quadrant q in {0,1} computes the even-x / odd-x conv outputs
   concurrently (32-row PE tiling), contracting K=10 rows: 9 im2col
   taps + a ones-row that adds the folded bias inside the matmul.
   Maxpool becomes max(even, odd) in x (one DVE op against the
   ACT-relu-evacuated odd parity) then a strided y-pair max; relu is
   folded into the max tree via max(a, relu(b)) == relu(max(a, b)).
 - conv2/conv3: 9-tap shifted-window accumulating matmuls over
   zero-padded SBUF activations; pooling uses an ACT relu+bias
   evacuation of the odd-x columns, a DVE scalar_tensor_tensor
   (even + bias) max odd, and a DVE y-pair max that writes the padded
   activation (or out3) directly - no separate bias/relu pass.
 - attention: per-image [64x36]^T@[64x1] matmuls -> PE transpose ->
   softmax -> broadcast-matmul with ones -> multiply+segmented reduce.
"""
import sys
if '/opt/trn_rl_repo' not in sys.path:
    sys.path.insert(0, '/opt/trn_rl_repo')

import contextlib
import numpy as np

import concourse.bass as bass
import concourse.mybir as mybir
import concourse.tile as tile

F32 = mybir.dt.float32
BF16 = mybir.dt.bfloat16
DT_MM = BF16
RELU = mybir.ActivationFunctionType.Relu
EXP = mybir.ActivationFunctionType.Exp
ADD = mybir.AluOpType.add
MAX = mybir.AluOpType.max

N_CORES = 8
B_TOTAL = 512
BPC = B_TOTAL // N_CORES   # 64 images per core
G = 8                      # images per group
NG = BPC // G              # 8 groups
EPS = 1e-5
GPS_YMAX = False           # offload conv1 y-max to gpsimd

_MAX_WAITS = 1  # this walrus build supports 1 sync-wait per instruction


def _install_tile_fixups():
    """The nix walrus here allows only ONE sync-wait per instruction; Tile's
    exit drain aggregates one wait per live proc onto a single Drain. Spread
    the waits across spare SP nops emitted just before the drain."""
    if getattr(tile.TileContext, '_drain_patched', False):
        return

    def _patched(self, tick_clock, wait_clock):
        from concourse.vector_clock import ScopedClock
        nc = self.nc
        nops = [nc.sync.nop().ins for _ in range(32)]
        drain_inst = nc.sync.drain()
        wait_clock.add_sem_waits(
            drain_inst.ins, ScopedClock({None: tick_clock.global_clock}))
        si = drain_inst.ins.sync_info
        if si is not None and len(si.on_wait) > _MAX_WAITS:
            waits = list(si.on_wait)
            drain_inst.ins.sync_info = mybir.SyncInfo(
                on_wait=waits[:_MAX_WAITS], on_update=list(si.on_update))
            rest = waits[_MAX_WAITS:]
            for i in range(0, len(rest), _MAX_WAITS):
                nops[i // _MAX_WAITS].sync_info = mybir.SyncInfo(
                    on_wait=rest[i:i + _MAX_WAITS], on_update=[])
        nc.all_engine_barrier()
        popped = nc._tile_sem_poison_stack.pop()
        assert popped is self._sem_poison
        nc.clear_and_free_semaphores(list(self.sems.allocated().values()))
        nc.all_engine_barrier()

    tile.TileContext._drain_and_barrier = _patched
    tile.TileContext._drain_patched = True


def _split_excess_waits(nc):
    """This walrus allows one sync-wait per instruction. Hoist excess waits
    onto same-engine nops inserted immediately before the instruction
    (sequential waits on one engine are equivalent to a combined wait)."""
    idx = 0
    for f in nc.m.functions:
        for b in f.blocks:
            out, changed = [], False
            for ins in b.instructions:
                si = ins.sync_info
                if si is not None and len(si.on_wait) > _MAX_WAITS:
                    waits = list(si.on_wait)
                    extra, keep = waits[:-_MAX_WAITS], waits[-_MAX_WAITS:]
                    for j in range(0, len(extra), _MAX_WAITS):
                        nop = mybir.InstNoOp(name=f"I-wsplit-{idx}")
                        idx += 1
                        nop.engine = ins.engine
                        nop.sync_info = mybir.SyncInfo(
                            on_wait=extra[j:j + _MAX_WAITS], on_update=[])
                        nc.register_instruction(nop, overwrite=True)
                        out.append(nop)
                    ins.sync_info = mybir.SyncInfo(
                        on_wait=keep, on_update=list(si.on_update))
                    changed = True
                out.append(ins)
            if changed:
                b.instructions = out


def _prep_weights(p):
    """Fold BN, collapse FC chain, lay out weights for the device program."""
    def fold(w, b, g, be, m, v):
        inv = (g / np.sqrt(v + EPS)).astype(np.float32)
        wf = (w * inv[:, None, None, None]).astype(np.float32)
        bf = ((b - m) * inv + be).astype(np.float32)
        return wf, bf

    w1, b1 = fold(p['conv1_w'], p['conv1_b'], p['bn1_g'], p['bn1_b'], p['bn1_m'], p['bn1_v'])
    w2, b2 = fold(p['conv2_w'], p['conv2_b'], p['bn2_g'], p['bn2_b'], p['bn2_m'], p['bn2_v'])
    w3, b3 = fold(p['conv3_w'], p['conv3_b'], p['bn3_g'], p['bn3_b'], p['bn3_m'], p['bn3_v'])

    # conv1 lhsT [128, 256]: rows 32q+k (k = 3*ky+kx) = w1[c, 0, ky, kx] for
    # q in {0,1} (even-x / odd-x PE quadrants); row 32q+9 = folded bias
    # (contracted against a ones-row in the im2col buffer).
    W1T = np.zeros((128, 256), np.float32)
    W1T[0:9, :] = w1.reshape(256, 9).T
    W1T[9, :] = b1
    # conv2 lhsT [128, 2304]: [p, t*256 + h*128 + m] = w2[m, 128h+p, t]
    W2T = np.ascontiguousarray(
        w2.reshape(128, 2, 128, 9).transpose(2, 3, 1, 0)  # [p, t, h, m]
    ).reshape(128, 2304)
    # conv3 lhsT [128, 576]: [p, t*64 + m] = w3[m, p, t]
    W3T = np.ascontiguousarray(
        w3.reshape(64, 128, 9).transpose(1, 2, 0)).reshape(128, 576)

    # FC chain collapse: q = out4 @ W_eff.T + b_eff
    fc1w, fc2w, attw = p['fc1_w'], p['fc2_w'], p['att_w']
    W_eff = (attw @ fc2w @ fc1w).astype(np.float32)          # [64, 2304]
    b_eff = (attw @ (fc2w @ p['fc1_b'] + p['fc2_b']) + p['att_b']).astype(np.float32)
    # WeT2 [64, 2304]: [c, hw*64 + m] = W_eff[m, c*36 + hw]
    WeT2 = np.ascontiguousarray(
        W_eff.reshape(64, 64, 36).transpose(1, 2, 0)).reshape(64, 2304)

    W3fT = np.ascontiguousarray(p['fc3_w'].T).astype(np.float32)  # [64, 7]
    fc3b_rep = np.broadcast_to(p['fc3_b'], (64, 7)).astype(np.float32).copy()

    b2c = b2.reshape(128, 1).astype(np.float32)
    b3c = b3.reshape(64, 1).astype(np.float32)
    beffc = b_eff.reshape(64, 1).astype(np.float32)

    return dict(W1T=W1T, W2T=W2T, W3T=W3T, WeT2=WeT2, W3fT=W3fT,
                fc3b_rep=fc3b_rep, b2c=b2c, b3c=b3c, beffc=beffc,
                Z=np.zeros((1, 18432), np.float32),
                ONESR=np.ones((1, 18432), np.float32),
                IDENT=np.eye(36, dtype=np.float32))


def _prep_x(x):
    """Deinterleave x columns by parity: out[b, q*1152 + y*24 + x'] =
    x[b, y, 2x'+q]. Lets the device build a parity-separated im2col with
    contiguous DMA runs (DMA final dims must be stride-1)."""
    xr = np.asarray(x, np.float32).reshape(-1, 48, 48)
    return np.stack([xr[:, :, 0::2], xr[:, :, 1::2]],
                    axis=1).reshape(-1, 2304)


def build_program(debug=False):
    """Build the per-core SPMD Bass program. Returns nc."""
    _install_tile_fixups()
    nc = bass.Bass("TRN2", target_bir_lowering=False, debug=False)

    x = nc.declare_dram_parameter("x", [BPC, 2304], DT_MM, isOutput=False)
    W1T = nc.declare_dram_parameter("W1T", [128, 256], DT_MM, isOutput=False)
    W2T = nc.declare_dram_parameter("W2T", [128, 2304], DT_MM, isOutput=False)
    W3T = nc.declare_dram_parameter("W3T", [128, 576], DT_MM, isOutput=False)
    WeT2 = nc.declare_dram_parameter("WeT2", [64, 2304], DT_MM, isOutput=False)
    W3fT = nc.declare_dram_parameter("W3fT", [64, 7], DT_MM, isOutput=False)
    fc3b = nc.declare_dram_parameter("fc3b_rep", [64, 7], F32, isOutput=False)
    b2c = nc.declare_dram_parameter("b2c", [128, 1], F32, isOutput=False)
    b3c = nc.declare_dram_parameter("b3c", [64, 1], F32, isOutput=False)
    beffc = nc.declare_dram_parameter("beffc", [64, 1], F32, isOutput=False)
    Z = nc.declare_dram_parameter("Z", [1, 18432], DT_MM, isOutput=False)
    ONESR = nc.declare_dram_parameter("ONESR", [1, 18432], DT_MM, isOutput=False)
    IDENT = nc.declare_dram_parameter("IDENT", [36, 36], F32, isOutput=False)
    out = nc.declare_dram_parameter("out", [BPC, 7], F32, isOutput=True)
    dbg = {}
    if debug:
        for nm, shp in [("dbg_act1_0", [128, G * 676]), ("dbg_act1_1", [128, G * 676]),
                        ("dbg_act2", [128, BPC * 196]), ("dbg_out3", [64, BPC * 36]),
                        ("dbg_q", [64, 64]), ("dbg_attn", [64, 36]),
                        ("dbg_gT", [64, 64]), ("dbg_sc", [36, 64])]:
            dbg[nm] = nc.declare_dram_parameter(nm, shp, F32, isOutput=True)

    r = lambda ap: ap

    with tile.TileContext(nc) as tc, contextlib.ExitStack() as ctx:
        wp = ctx.enter_context(tc.tile_pool(name="weights", bufs=1))
        ap_pool = ctx.enter_context(tc.tile_pool(name="acts", bufs=1))
        cp = ctx.enter_context(tc.tile_pool(name="im2col", bufs=2))
        t1p = ctx.enter_context(tc.tile_pool(name="t1", bufs=3))
        e2p = ctx.enter_context(tc.tile_pool(name="ev2", bufs=6))
        e3p = ctx.enter_context(tc.tile_pool(name="ev3", bufs=3))

        # ---- group-0 input chain first, ahead of the bulk weight DMAs ----
        # xpq: zero-padded x staged as two x-parity planes per image:
        # cols p'*1250 + y_p*25 + x'_p with padded x_p = 2*x'_p + p'.
        # Host-deinterleaved x (see _prep_x) makes these loads contiguous.
        xpqs = [ap_pool.tile([8, 2500], DT_MM, tag=f"xpq{pp}",
                             name=f"xpq{pp}") for pp in range(2)]
        for pp in range(2):
            nc.sync.dma_start(out=xpqs[pp][:],
                              in_=Z[:, :2500].to_broadcast((8, 2500)))

        def load_xpq(g):
            xpqv = xpqs[g % 2][:].rearrange("p (pp y x) -> p pp y x", y=50, x=25)
            xg = x[G * g:G * (g + 1), :].rearrange("b (qq c) -> b qq c", qq=2)
            nc.gpsimd.dma_start(out=xpqv[0:G, 0, 1:49, 1:25], in_=xg[:, 1, :])
            nc.gpsimd.dma_start(out=xpqv[0:G, 1, 1:49, 0:24], in_=xg[:, 0, :])

        def load_taps(g, imt):
            """im2col taps: row k, parity-q block <- padded parity plane.
            Output-x parity q tap (dy,dx) reads plane (q+dx)%2 shifted.
            gpsimd (SWDGE): ~4x faster than HWDGE for this descriptor
            pattern (measured)."""
            xpqv = xpqs[g % 2][:].rearrange("p (pp y x) -> p pp y x", y=50, x=25)
            ivk = imt.rearrange("p (qq c) -> p qq c", qq=2)
            for q in range(2):
                for k in range(9):
                    dy, dx = divmod(k, 3)
                    nc.gpsimd.dma_start(
                        out=ivk[k:k + 1, q, :],
                        in_=xpqv[0:G, (q + dx) % 2, dy:dy + 48,
                                 (q + dx) // 2:(q + dx) // 2 + 24])

        # Persistent im2col tiles (group-parity double buffer), [128,
        # 2*G*1152]: rows 0-8 hold tap k (rewritten per group), row 9 the
        # ones-row contracting the folded bias, rows 10-64 zero. K=65
        # keeps conv1 matmuls in the full 128x128 PE tile mode: mixing
        # 32-row-tiled matmuls with conv2's full-mode ones halves the PE
        # clock around every switch (measured: 288-col MMs at 240ns).
        imts = [ap_pool.tile([128, G * 2304], DT_MM, tag=f"imt{pp}",
                             name=f"imt{pp}") for pp in range(2)]
        load_xpq(0)
        load_taps(0, imts[0][:])

        # ---- load weights (ahead of the bulk zero-fills: the first
        # matmuls need them; multi-MB fills would delay them ~40us) ----
        w1t = wp.tile([128, 256], DT_MM)
        nc.sync.dma_start(out=w1t[:], in_=W1T[:])
        w2t = wp.tile([128, 2304], DT_MM)
        nc.sync.dma_start(out=w2t[:], in_=W2T[:])
        w3t = wp.tile([128, 576], DT_MM)
        nc.sync.dma_start(out=w3t[:], in_=W3T[:])
        wet = wp.tile([64, 2304], DT_MM)
        nc.sync.dma_start(out=wet[:], in_=WeT2[:])
        w3f = wp.tile([64, 7], DT_MM)
        nc.sync.dma_start(out=w3f[:], in_=W3fT[:])
        fc3b_t = wp.tile([64, 7], F32)
        nc.sync.dma_start(out=fc3b_t[:], in_=fc3b[:])
        b2t = wp.tile([128, 1], F32)
        nc.sync.dma_start(out=b2t[:], in_=b2c[:])
        b3t = wp.tile([64, 1], F32)
        nc.sync.dma_start(out=b3t[:], in_=b3c[:])
        bet = wp.tile([64, 1], F32)
        nc.sync.dma_start(out=bet[:], in_=beffc[:])
        ident = wp.tile([36, 36], F32)
        nc.sync.dma_start(out=ident[:], in_=IDENT[:])
        ones1 = wp.tile([1, 64], DT_MM)
        nc.sync.dma_start(out=ones1[:], in_=ONESR[:, 0:64])

        # one-time fills, chunked to ~0.5MB so they don't monopolize the
        # DMA fabric while taps/acts stream in
        for t in imts:
            nc.sync.dma_start(out=t[9:10, :], in_=ONESR[:, :G * 2304])
            for c in range(4):
                nc.sync.dma_start(
                    out=t[10:65, 4608 * c:4608 * (c + 1)],
                    in_=Z[:, :4608].to_broadcast((55, 4608)))

        # ---- persistent activation buffers ----
        act1 = [[ap_pool.tile([128, G * 676], DT_MM, tag=f"act1_{pp}_{h}",
                              name=f"act1_{pp}_{h}") for h in range(2)]
                for pp in range(2)]
        act2 = ap_pool.tile([128, BPC * 196], DT_MM)
        out3 = ap_pool.tile([64, BPC * 36], DT_MM)
        def fill_acts():
            for pp in range(2):
                for h in range(2):
                    for c in range(2):
                        nc.sync.dma_start(
                            out=act1[pp][h][:, 2704 * c:2704 * (c + 1)],
                            in_=Z[:, :2704].to_broadcast((128, 2704)))
            for c in range(4):
                nc.sync.dma_start(
                    out=act2[:, 3136 * c:3136 * (c + 1)],
                    in_=Z[:, :3136].to_broadcast((128, 3136)))

        with contextlib.ExitStack() as cctx:
            ps1 = cctx.enter_context(tc.tile_pool(name="ps1", bufs=1, space="PSUM"))
            ps2 = cctx.enter_context(tc.tile_pool(name="ps2", bufs=2, space="PSUM"))

            # PE warm-up: ~2.5us of dummy matmuls as soon as the weights
            # land, so the HAM clock gate reaches 8/8 before real work
            # (cold MMs run at 1.2 GHz instead of 2.4).
            psW = ps1.tile([128, 1536], F32, tag="psE", name="psW")
            for i in range(12):
                nc.tensor.matmul(out=psW[:, 0:512], lhsT=r(w1t[0:65, 0:128]),
                                 rhs=r(w2t[0:65, 0:512]), start=True, stop=True)

            def conv1_h(g, ci, imt, h):
                """conv1 half h of one image: x-parity matmuls + pooled/
                relu'd write to act1."""
                iv = imt.rearrange("p (qq b y x) -> p qq b y x",
                                   qq=2, y=48, x=24)
                if True:
                    # psum layout: 3 banks x (16 y-rows x 24 x-cols = 384)
                    psE = ps1.tile([128, 1536], F32, tag="psE", name="psE")
                    psO = ps1.tile([128, 1536], F32, tag="psO", name="psO")
                    for bk in range(3):
                        for q, ps in ((0, psE), (1, psO)):
                            nc.tensor.matmul(
                                out=ps[:, 512 * bk:512 * bk + 384],
                                lhsT=r(w1t[0:65, 128 * h:128 * (h + 1)]),
                                rhs=r(iv[0:65, q, ci,
                                         16 * bk:16 * bk + 16, :]),
                                start=True, stop=True)
                    psEv = psE[:].rearrange("p (k c) -> p k c", c=512)[:, :, 0:384]
                    psOv = psO[:].rearrange("p (k c) -> p k c", c=512)[:, :, 0:384]
                    # odd parity: relu-evac (bias already in psum via ones-row)
                    oddr = t1p.tile([128, 1152], DT_MM, tag="oddr", name="oddr")
                    oddv = oddr[:].rearrange("p (k c) -> p k c", c=384)
                    nc.scalar.activation(out=oddv, in_=psOv, func=RELU)
                    # max(even, relu(odd)) == relu(max(even, odd)) elementwise
                    m1 = t1p.tile([128, 1152], DT_MM, tag="m1", name="m1")
                    m1v3 = m1[:].rearrange("p (k c) -> p k c", c=384)
                    nc.vector.tensor_max(m1v3, psEv, oddv)
                    # y-pair max -> act1 padded interior (image ci)
                    m1v = m1[:].rearrange("p (y x) -> p y x", x=24)
                    dst = act1[g % 2][h][:].rearrange(
                        "p (b y x) -> p b y x", y=26, x=26)[:, ci, 1:25, 1:25]
                    eng = nc.gpsimd if GPS_YMAX else nc.vector
                    eng.tensor_max(dst, m1v[:, 0:48:2, :], m1v[:, 1:48:2, :])

            def conv2_rr(g, bb, rr):
                """conv2 for image bb of group g, output row-half rr."""
                a1v = [act1[g % 2][h][:].rearrange(
                    "p (b y x) -> p b y x", y=26, x=26) for h in range(2)]
                a2v = act2[:].rearrange("p (b y x) -> p b y x", y=14, x=14)
                if True:
                    ps = ps2.tile([128, 288], F32, tag="ps2")
                    i = 0
                    for t in range(9):
                        dy, dx = divmod(t, 3)
                        for h in range(2):
                            nc.tensor.matmul(
                                out=ps[:],
                                lhsT=r(w2t[:, (t * 2 + h) * 128:(t * 2 + h + 1) * 128]),
                                rhs=r(a1v[h][:, bb, 12 * rr + dy:12 * rr + dy + 12,
                                             dx:dx + 24]),
                                start=(i == 0), stop=(i == 17))
                            i += 1
                    psv = ps[:].rearrange("p (y x) -> p y x", x=24)
                    todd2 = e2p.tile([128, 144], DT_MM, tag="todd2")
                    todd2v = todd2[:].rearrange("p (y x) -> p y x", x=12)
                    nc.scalar.activation(out=todd2v, in_=psv[:, :, 1:24:2],
                                         func=RELU, bias=b2t[:])
                    t1c = e2p.tile([128, 144], DT_MM, tag="t1c2")
                    t1cv = t1c[:].rearrange("p (y x) -> p y x", x=12)
                    nc.vector.scalar_tensor_tensor(
                        out=t1cv, in0=psv[:, :, 0:24:2], scalar=b2t[:],
                        in1=todd2v, op0=ADD, op1=MAX)
                    dst = a2v[:, g * G + bb, 6 * rr + 1:6 * rr + 7, 1:13]
                    nc.vector.tensor_max(dst, t1cv[:, 0:12:2, :], t1cv[:, 1:12:2, :])

            # ---- group loop: conv1(g) interleaved with conv2(g-1) ----
            prev_g = None
            for g in range(NG):
                imt = imts[g % 2]
                if g == 0:
                    fill_acts()   # overlaps with group-0 im2col + matmuls
                else:
                    load_xpq(g)
                    load_taps(g, imt[:])
                for ci in range(G):
                    # fine interleave: each conv2 chain fills the PE while
                    # the preceding conv1 half's psum drains through the
                    # pool engines
                    for h in range(2):
                        if prev_g is not None:
                            conv2_rr(prev_g, ci, h)
                        conv1_h(g, ci, imt[:], h)
                prev_g = g
            for ci in range(G):
                for rr in range(2):
                    conv2_rr(prev_g, ci, rr)

        # ---- conv3 (all groups done; act2 complete) ----
        with contextlib.ExitStack() as cctx:
            ps3 = cctx.enter_context(tc.tile_pool(name="ps3", bufs=3, space="PSUM"))
            psq = cctx.enter_context(tc.tile_pool(name="psq", bufs=1, space="PSUM"))
            pssc = cctx.enter_context(tc.tile_pool(name="pssc", bufs=1, space="PSUM"))
            psT = cctx.enter_context(tc.tile_pool(name="psT", bufs=1, space="PSUM"))

            a2v = act2[:].rearrange("p (b y x) -> p b y x", y=14, x=14)
            o3v = out3[:].rearrange("p (b hw) -> p b hw", hw=36)
            for t in range(32):  # image pairs
                ps = ps3.tile([64, 288], F32, tag="ps3")
                for k in range(9):
                    dy, dx = divmod(k, 3)
                    nc.tensor.matmul(
                        out=ps[:],
                        lhsT=r(w3t[:, 64 * k:64 * (k + 1)]),
                        rhs=r(a2v[:, 2 * t:2 * t + 2, dy:dy + 12, dx:dx + 12]),
                        start=(k == 0), stop=(k == 8))
                psv = ps[:].rearrange("p (b y x) -> p b y x", y=12, x=12)
                todd3 = e3p.tile([64, 144], DT_MM, tag="todd3")
                todd3v = todd3[:].rearrange("p (b y x) -> p b y x", y=12, x=6)
                nc.scalar.activation(out=todd3v, in_=psv[:, :, :, 1:12:2],
                                     func=RELU, bias=b3t[:])
                t1c = e3p.tile([64, 144], DT_MM, tag="t1c3")
                t1cv = t1c[:].rearrange("p (b y x) -> p b y x", y=12, x=6)
                nc.vector.scalar_tensor_tensor(
                    out=t1cv, in0=psv[:, :, :, 0:12:2], scalar=b3t[:],
                    in1=todd3v, op0=ADD, op1=MAX)
                nc.vector.tensor_max(
                    o3v[:, 2 * t:2 * t + 2, :].rearrange("p b (y x) -> p b y x", x=6),
                    t1cv[:, :, 0:12:2, :], t1cv[:, :, 1:12:2, :])

            # ---- q = W_eff @ out4 + b_eff : accumulate over hw ----
            psq_t = psq.tile([64, 64], F32)
            for hw in range(36):
                nc.tensor.matmul(
                    out=psq_t[:],
                    lhsT=r(wet[:, 64 * hw:64 * (hw + 1)]),
                    rhs=r(out3[:, hw:2304:36]),
                    start=(hw == 0), stop=(hw == 35))
            q_sb = ap_pool.tile([64, 64], DT_MM)
            nc.vector.tensor_scalar_add(q_sb[:], psq_t[:], bet[:])

            # ---- scores: per-image matmuls -> [36, 64] psum ----
            pssc_t = pssc.tile([36, 64], F32)
            for b in range(BPC):
                nc.tensor.matmul(
                    out=pssc_t[:, b:b + 1],
                    lhsT=out3[:, 36 * b:36 * (b + 1)],
                    rhs=q_sb[:, b:b + 1],
                    start=True, stop=True)
            sc_sb = ap_pool.tile([36, 64], F32)
            nc.vector.tensor_copy(sc_sb[:], pssc_t[:])
            psT_t = psT.tile([64, 36], F32)
            nc.tensor.transpose(psT_t[:], sc_sb[:], ident[:])

            # ---- softmax over hw (free dim) ----
            mx = ap_pool.tile([64, 1], F32)
            nc.vector.tensor_reduce(out=mx[:], in_=psT_t[:],
                                    op=mybir.AluOpType.max,
                                    axis=mybir.AxisListType.X)
            nmx = ap_pool.tile([64, 1], F32)
            nc.vector.tensor_scalar_mul(nmx[:], mx[:], -1.0)
            e_t = ap_pool.tile([64, 36], F32)
            nc.scalar.activation(out=e_t[:], in_=psT_t[:], func=EXP, bias=nmx[:])
            z = ap_pool.tile([64, 1], F32)
            nc.vector.tensor_reduce(out=z[:], in_=e_t[:],
                                    op=mybir.AluOpType.add,
                                    axis=mybir.AxisListType.X)
            rz = ap_pool.tile([64, 1], F32)
            nc.vector.reciprocal(rz[:], z[:])
            attn = ap_pool.tile([64, 36], DT_MM)
            nc.vector.tensor_scalar_mul(attn[:], e_t[:], rz[:])
            if debug:
                nc.gpsimd.dma_start(out=dbg["dbg_act1_0"][:], in_=act1[0][0][:])
                nc.gpsimd.dma_start(out=dbg["dbg_act1_1"][:], in_=act1[0][1][:])
                nc.gpsimd.dma_start(out=dbg["dbg_act2"][:], in_=act2[:])
                nc.gpsimd.dma_start(out=dbg["dbg_out3"][:], in_=out3[:])
                nc.gpsimd.dma_start(out=dbg["dbg_q"][:], in_=q_sb[:])
                nc.gpsimd.dma_start(out=dbg["dbg_attn"][:], in_=attn[:])
                nc.sync.dma_start(out=dbg["dbg_sc"][:], in_=sc_sb[:])

        # ---- g_mod + fc3 ----
        with contextlib.ExitStack() as cctx:
            psab = cctx.enter_context(tc.tile_pool(name="psab", bufs=1, space="PSUM"))
            psf = cctx.enter_context(tc.tile_pool(name="psf", bufs=1, space="PSUM"))

            attn_flat = ap_pool.tile([1, 2304], DT_MM)
            nc.sync.dma_start(out=attn_flat[:], in_=attn[:])
            psab_t = psab.tile([64, 2304], F32)
            for c in range(5):
                lo = 512 * c
                hi = min(lo + 512, 2304)
                nc.tensor.matmul(out=psab_t[:, lo:hi], lhsT=r(ones1[:]),
                                 rhs=r(attn_flat[:, lo:hi]), start=True, stop=True)
            # in-place: out3 is not needed after this product
            nc.vector.tensor_mul(out3[:], out3[:], psab_t[:])
            gT = ap_pool.tile([64, 64], DT_MM)
            with nc.allow_low_precision(reason="bf16 output of attn-weighted sum"):
                nc.vector.tensor_reduce(
                    out=gT[:], in_=out3[:].rearrange("p (b hw) -> p b hw", hw=36),
                    op=mybir.AluOpType.add, axis=mybir.AxisListType.X)

            if debug:
                nc.gpsimd.dma_start(out=dbg["dbg_gT"][:], in_=gT[:])
            psf_t = psf.tile([64, 7], F32)
            nc.tensor.matmul(out=psf_t[:], lhsT=gT[:],
                             rhs=w3f[:], start=True, stop=True)
            out_sb = ap_pool.tile([64, 7], F32)
            nc.vector.tensor_add(out_sb[:], psf_t[:], fc3b_t[:])
            nc.sync.dma_start(out=out[:], in_=out_sb[:])

    _split_excess_waits(nc)
    return nc


def kernel(**inputs):
    from concourse.bass_utils import run_bass_kernel_spmd

    w = _prep_weights({k: np.asarray(v, np.float32) for k, v in inputs.items()
                       if k != 'x'})
    npdt = mybir.dt.np(DT_MM)
    for k in ('W1T', 'W2T', 'W3T', 'WeT2', 'W3fT', 'Z', 'ONESR'):
        w[k] = w[k].astype(npdt)
    xs = _prep_x(inputs['x']).astype(npdt)

    nc = build_program()
    in_maps = []
    for c in range(N_CORES):
        m = {'x': np.ascontiguousarray(xs[BPC * c:BPC * (c + 1)])}
        m.update({k: v for k, v in w.items()})
        in_maps.append(m)
    res = run_bass_kernel_spmd(nc, in_maps, list(range(N_CORES)))
    outs = [res.results[c]['out'] for c in range(N_CORES)]
    return np.concatenate(outs, axis=0).astype(np.float32)


if __name__ == '__main__':
    rng = np.random.default_rng(0)
    fake = {
        'x': rng.standard_normal((512, 1, 48, 48), dtype=np.float32),
        'conv1_w': rng.standard_normal((256, 1, 3, 3), dtype=np.float32) * 0.05,
        'conv1_b': np.zeros(256, np.float32),
        'bn1_g': np.ones(256, np.float32), 'bn1_b': np.zeros(256, np.float32),
        'bn1_m': np.zeros(256, np.float32), 'bn1_v': np.ones(256, np.float32),
        'conv2_w': rng.standard_normal((128, 256, 3, 3), dtype=np.float32) * 0.05,
        'conv2_b': np.zeros(128, np.float32),
        'bn2_g': np.ones(128, np.float32), 'bn2_b': np.zeros(128, np.float32),
        'bn2_m': np.zeros(128, np.float32), 'bn2_v': np.ones(128, np.float32),
        'conv3_w': rng.standard_normal((64, 128, 3, 3), dtype=np.float32) * 0.05,
        'conv3_b': np.zeros(64, np.float32),
        'bn3_g': np.ones(64, np.float32), 'bn3_b': np.zeros(64, np.float32),
        'bn3_m': np.zeros(64, np.float32), 'bn3_v': np.ones(64, np.float32),
        'fc1_w': rng.standard_normal((512, 2304), dtype=np.float32) * 0.05,
        'fc1_b': np.zeros(512, np.float32),
        'fc2_w': rng.standard_normal((256, 512), dtype=np.float32) * 0.05,
        'fc2_b': np.zeros(256, np.float32),
        'att_w': rng.standard_normal((64, 256), dtype=np.float32) * 0.05,
        'att_b': np.zeros(64, np.float32),
        'fc3_w': rng.standard_normal((7, 64), dtype=np.float32) * 0.05,
        'fc3_b': np.zeros(7, np.float32),
    }
    print(kernel(**fake).shape)


# revision 24
# speedup vs baseline: 1.1893x; 1.1893x over previous
"""Trainium2 Bass kernel for nn_BaselineModel_80796924772520 (dense_cnn).

Self-contained: kernel(**inputs) -> np.ndarray [512, 7] float32.

Strategy: pure data parallelism over 8 NeuronCores (64 images each).
 - BN folded into conv weights/biases on host (eval-mode BN is affine).
 - fc1/fc2/att collapse into one linear map W_eff [64, 2304] on host
   (reference has no nonlinearity between them).
 - conv1 (C_in=1, K=9): x-parity decomposition on two PE row-quadrants.
   Quadrant q in {0,1} computes the even-x / odd-x conv outputs
   concurrently (32-row PE tiling), contracting K=10 rows: 9 im2col
   taps + a ones-row that adds the folded bias inside the matmul.
   Maxpool becomes max(even, odd) in x (one DVE op against the
   ACT-relu-evacuated odd parity) then a strided y-pair max; relu is
   folded into the max tree via max(a, relu(b)) == relu(max(a, b)).
 - conv2/conv3: 9-tap shifted-window accumulating matmuls over
   zero-padded SBUF activations; pooling uses an ACT relu+bias
   evacuation of the odd-x columns, a DVE scalar_tensor_tensor
   (even + bias) max odd, and a DVE y-pair max that writes the padded
   activation (or out3) directly - no separate bias/relu pass.
 - attention: per-image [64x36]^T@[64x1] matmuls -> PE transpose ->
   softmax -> broadcast-matmul with ones -> multiply+segmented reduce.
"""
import sys
if '/opt/trn_rl_repo' not in sys.path:
    sys.path.insert(0, '/opt/trn_rl_repo')

import contextlib
import numpy as np

import concourse.bass as bass
import concourse.mybir as mybir
import concourse.tile as tile

F32 = mybir.dt.float32
BF16 = mybir.dt.bfloat16
DT_MM = BF16
RELU = mybir.ActivationFunctionType.Relu
EXP = mybir.ActivationFunctionType.Exp
ADD = mybir.AluOpType.add
MAX = mybir.AluOpType.max

N_CORES = 8
B_TOTAL = 512
BPC = B_TOTAL // N_CORES   # 64 images per core
G = 8                      # images per group
NG = BPC // G              # 8 groups
EPS = 1e-5
GPS_YMAX = False           # offload conv1 y-max to gpsimd

_MAX_WAITS = 1  # this walrus build supports 1 sync-wait per instruction


def _install_tile_fixups():
    """The nix walrus here allows only ONE sync-wait per instruction; Tile's
    exit drain aggregates one wait per live proc onto a single Drain. Spread
    the waits across spare SP nops emitted just before the drain."""
    if getattr(tile.TileContext, '_drain_patched', False):
        return

    def _patched(self, tick_clock, wait_clock):
        from concourse.vector_clock import ScopedClock
        nc = self.nc
        nops = [nc.sync.nop().ins for _ in range(32)]
        drain_inst = nc.sync.drain()
        wait_clock.add_sem_waits(
            drain_inst.ins, ScopedClock({None: tick_clock.global_clock}))
        si = drain_inst.ins.sync_info
        if si is not None and len(si.on_wait) > _MAX_WAITS:
            waits = list(si.on_wait)
            drain_inst.ins.sync_info = mybir.SyncInfo(
                on_wait=waits[:_MAX_WAITS], on_update=list(si.on_update))
            rest = waits[_MAX_WAITS:]
            for i in range(0, len(rest), _MAX_WAITS):
                nops[i // _MAX_WAITS].sync_info = mybir.SyncInfo(
                    on_wait=rest[i:i + _MAX_WAITS], on_update=[])
        nc.all_engine_barrier()
        popped = nc._tile_sem_poison_stack.pop()
        assert popped is self._sem_poison
        nc.clear_and_free_semaphores(list(self.sems.allocated().values()))
        nc.all_engine_barrier()

    tile.TileContext._drain_and_barrier = _patched
    tile.TileContext._drain_patched = True


def _split_excess_waits(nc):
    """This walrus allows one sync-wait per instruction. Hoist excess waits
    onto same-engine nops inserted immediately before the instruction
    (sequential waits on one engine are equivalent to a combined wait)."""
    idx = 0
    for f in nc.m.functions:
        for b in f.blocks:
            out, changed = [], False
            for ins in b.instructions:
                si = ins.sync_info
                if si is not None and len(si.on_wait) > _MAX_WAITS:
                    waits = list(si.on_wait)
                    extra, keep = waits[:-_MAX_WAITS], waits[-_MAX_WAITS:]
                    for j in range(0, len(extra), _MAX_WAITS):
                        nop = mybir.InstNoOp(name=f"I-wsplit-{idx}")
                        idx += 1
                        nop.engine = ins.engine
                        nop.sync_info = mybir.SyncInfo(
                            on_wait=extra[j:j + _MAX_WAITS], on_update=[])
                        nc.register_instruction(nop, overwrite=True)
                        out.append(nop)
                    ins.sync_info = mybir.SyncInfo(
                        on_wait=keep, on_update=list(si.on_update))
                    changed = True
                out.append(ins)
            if changed:
                b.instructions = out


def _prep_weights(p):
    """Fold BN, collapse FC chain, lay out weights for the device program."""
    def fold(w, b, g, be, m, v):
        inv = (g / np.sqrt(v + EPS)).astype(np.float32)
        wf = (w * inv[:, None, None, None]).astype(np.float32)
        bf = ((b - m) * inv + be).astype(np.float32)
        return wf, bf

    w1, b1 = fold(p['conv1_w'], p['conv1_b'], p['bn1_g'], p['bn1_b'], p['bn1_m'], p['bn1_v'])
    w2, b2 = fold(p['conv2_w'], p['conv2_b'], p['bn2_g'], p['bn2_b'], p['bn2_m'], p['bn2_v'])
    w3, b3 = fold(p['conv3_w'], p['conv3_b'], p['bn3_g'], p['bn3_b'], p['bn3_m'], p['bn3_v'])

    # conv1 lhsT [128, 256]: rows 32q+k (k = 3*ky+kx) = w1[c, 0, ky, kx] for
    # q in {0,1} (even-x / odd-x PE quadrants); row 32q+9 = folded bias
    # (contracted against a ones-row in the im2col buffer).
    W1T = np.zeros((128, 256), np.float32)
    W1T[0:9, :] = w1.reshape(256, 9).T
    W1T[9, :] = b1
    # conv2 lhsT [128, 2304]: [p, t*256 + h*128 + m] = w2[m, 128h+p, t]
    W2T = np.ascontiguousarray(
        w2.reshape(128, 2, 128, 9).transpose(2, 3, 1, 0)  # [p, t, h, m]
    ).reshape(128, 2304)
    # conv3 lhsT [128, 576]: [p, t*64 + m] = w3[m, p, t]
    W3T = np.ascontiguousarray(
        w3.reshape(64, 128, 9).transpose(1, 2, 0)).reshape(128, 576)

    # FC chain collapse: q = out4 @ W_eff.T + b_eff
    fc1w, fc2w, attw = p['fc1_w'], p['fc2_w'], p['att_w']
    W_eff = (attw @ fc2w @ fc1w).astype(np.float32)          # [64, 2304]
    b_eff = (attw @ (fc2w @ p['fc1_b'] + p['fc2_b']) + p['att_b']).astype(np.float32)
    # WeT2 [64, 2304]: [c, hw*64 + m] = W_eff[m, c*36 + hw]
    WeT2 = np.ascontiguousarray(
        W_eff.reshape(64, 64, 36).transpose(1, 2, 0)).reshape(64, 2304)

    W3fT = np.ascontiguousarray(p['fc3_w'].T).astype(np.float32)  # [64, 7]
    fc3b_rep = np.broadcast_to(p['fc3_b'], (64, 7)).astype(np.float32).copy()

    b2c = b2.reshape(128, 1).astype(np.float32)
    b3c = b3.reshape(64, 1).astype(np.float32)
    beffc = b_eff.reshape(64, 1).astype(np.float32)

    return dict(W1T=W1T, W2T=W2T, W3T=W3T, WeT2=WeT2, W3fT=W3fT,
                fc3b_rep=fc3b_rep, b2c=b2c, b3c=b3c, beffc=beffc,
                Z=np.zeros((1, 18432), np.float32),
                ONESR=np.ones((1, 18432), np.float32),
                IDENT=np.eye(36, dtype=np.float32))


def _prep_x(x):
    """Deinterleave x columns by parity: out[b, q*1152 + y*24 + x'] =
    x[b, y, 2x'+q]. Lets the device build a parity-separated im2col with
    contiguous DMA runs (DMA final dims must be stride-1)."""
    xr = np.asarray(x, np.float32).reshape(-1, 48, 48)
    return np.stack([xr[:, :, 0::2], xr[:, :, 1::2]],
                    axis=1).reshape(-1, 2304)


def build_program(debug=False):
    """Build the per-core SPMD Bass program. Returns nc."""
    _install_tile_fixups()
    nc = bass.Bass("TRN2", target_bir_lowering=False, debug=False)

    x = nc.declare_dram_parameter("x", [BPC, 2304], DT_MM, isOutput=False)
    W1T = nc.declare_dram_parameter("W1T", [128, 256], DT_MM, isOutput=False)
    W2T = nc.declare_dram_parameter("W2T", [128, 2304], DT_MM, isOutput=False)
    W3T = nc.declare_dram_parameter("W3T", [128, 576], DT_MM, isOutput=False)
    WeT2 = nc.declare_dram_parameter("WeT2", [64, 2304], DT_MM, isOutput=False)
    W3fT = nc.declare_dram_parameter("W3fT", [64, 7], DT_MM, isOutput=False)
    fc3b = nc.declare_dram_parameter("fc3b_rep", [64, 7], F32, isOutput=False)
    b2c = nc.declare_dram_parameter("b2c", [128, 1], F32, isOutput=False)
    b3c = nc.declare_dram_parameter("b3c", [64, 1], F32, isOutput=False)
    beffc = nc.declare_dram_parameter("beffc", [64, 1], F32, isOutput=False)
    Z = nc.declare_dram_parameter("Z", [1, 18432], DT_MM, isOutput=False)
    ONESR = nc.declare_dram_parameter("ONESR", [1, 18432], DT_MM, isOutput=False)
    IDENT = nc.declare_dram_parameter("IDENT", [36, 36], F32, isOutput=False)
    out = nc.declare_dram_parameter("out", [BPC, 7], F32, isOutput=True)
    dbg = {}
    if debug:
        for nm, shp in [("dbg_act1_0", [128, G * 676]), ("dbg_act1_1", [128, G * 676]),
                        ("dbg_act2", [128, BPC * 196]), ("dbg_out3", [64, BPC * 36]),
                        ("dbg_q", [64, 64]), ("dbg_attn", [64, 36]),
                        ("dbg_gT", [64, 64]), ("dbg_sc", [36, 64])]:
            dbg[nm] = nc.declare_dram_parameter(nm, shp, F32, isOutput=True)

    r = lambda ap: ap

    with tile.TileContext(nc) as tc, contextlib.ExitStack() as ctx:
        wp = ctx.enter_context(tc.tile_pool(name="weights", bufs=1))
        ap_pool = ctx.enter_context(tc.tile_pool(name="acts", bufs=1))
        cp = ctx.enter_context(tc.tile_pool(name="im2col", bufs=2))
        t1p = ctx.enter_context(tc.tile_pool(name="t1", bufs=3))
        e2p = ctx.enter_context(tc.tile_pool(name="ev2", bufs=6))
        e3p = ctx.enter_context(tc.tile_pool(name="ev3", bufs=3))

        # ---- group-0 input chain first, ahead of the bulk weight DMAs ----
        # xpq: zero-padded x staged as two x-parity planes per image:
        # cols p'*1250 + y_p*25 + x'_p with padded x_p = 2*x'_p + p'.
        # Host-deinterleaved x (see _prep_x) makes these loads contiguous.
        xpqs = [ap_pool.tile([8, 2500], DT_MM, tag=f"xpq{pp}",
                             name=f"xpq{pp}") for pp in range(2)]
        for pp in range(2):
            nc.sync.dma_start(out=xpqs[pp][:],
                              in_=Z[:, :2500].to_broadcast((8, 2500)))

        def load_xpq(g):
            xpqv = xpqs[g % 2][:].rearrange("p (pp y x) -> p pp y x", y=50, x=25)
            xg = x[G * g:G * (g + 1), :].rearrange("b (qq c) -> b qq c", qq=2)
            nc.gpsimd.dma_start(out=xpqv[0:G, 0, 1:49, 1:25], in_=xg[:, 1, :])
            nc.gpsimd.dma_start(out=xpqv[0:G, 1, 1:49, 0:24], in_=xg[:, 0, :])

        def load_taps(g, imt):
            """im2col taps: row k, parity-q block <- padded parity plane.
            Output-x parity q tap (dy,dx) reads plane (q+dx)%2 shifted.
            gpsimd (SWDGE): ~4x faster than HWDGE for this descriptor
            pattern (measured)."""
            xpqv = xpqs[g % 2][:].rearrange("p (pp y x) -> p pp y x", y=50, x=25)
            ivk = imt.rearrange("p (qq c) -> p qq c", qq=2)
            for q in range(2):
                for k in range(9):
                    dy, dx = divmod(k, 3)
                    nc.gpsimd.dma_start(
                        out=ivk[k:k + 1, q, :],
                        in_=xpqv[0:G, (q + dx) % 2, dy:dy + 48,
                                 (q + dx) // 2:(q + dx) // 2 + 24])

        # Persistent im2col tiles (group-parity double buffer), [128,
        # 2*G*1152]: rows 0-8 hold tap k (rewritten per group), row 9 the
        # ones-row contracting the folded bias, rows 10-64 zero. K=65
        # keeps conv1 matmuls in the full 128x128 PE tile mode: mixing
        # 32-row-tiled matmuls with conv2's full-mode ones halves the PE
        # clock around every switch (measured: 288-col MMs at 240ns).
        imts = [ap_pool.tile([128, G * 2304], DT_MM, tag=f"imt{pp}",
                             name=f"imt{pp}") for pp in range(2)]
        load_xpq(0)
        load_taps(0, imts[0][:])

        # ---- load weights (ahead of the bulk zero-fills: the first
        # matmuls need them; multi-MB fills would delay them ~40us) ----
        w1t = wp.tile([128, 256], DT_MM)
        nc.sync.dma_start(out=w1t[:], in_=W1T[:])
        w2t = wp.tile([128, 2304], DT_MM)
        nc.sync.dma_start(out=w2t[:], in_=W2T[:])
        w3t = wp.tile([128, 576], DT_MM)
        nc.sync.dma_start(out=w3t[:], in_=W3T[:])
        wet = wp.tile([64, 2304], DT_MM)
        nc.sync.dma_start(out=wet[:], in_=WeT2[:])
        w3f = wp.tile([64, 7], DT_MM)
        nc.sync.dma_start(out=w3f[:], in_=W3fT[:])
        fc3b_t = wp.tile([64, 7], F32)
        nc.sync.dma_start(out=fc3b_t[:], in_=fc3b[:])
        b2t = wp.tile([128, 1], F32)
        nc.sync.dma_start(out=b2t[:], in_=b2c[:])
        b3t = wp.tile([64, 1], F32)
        nc.sync.dma_start(out=b3t[:], in_=b3c[:])
        bet = wp.tile([64, 1], F32)
        nc.sync.dma_start(out=bet[:], in_=beffc[:])
        ident = wp.tile([36, 36], F32)
        nc.sync.dma_start(out=ident[:], in_=IDENT[:])
        ones1 = wp.tile([1, 64], DT_MM)
        nc.sync.dma_start(out=ones1[:], in_=ONESR[:, 0:64])


        # ---- persistent activation buffers ----
        act1 = [[ap_pool.tile([128, G * 676], DT_MM, tag=f"act1_{pp}_{h}",
                              name=f"act1_{pp}_{h}") for h in range(2)]
                for pp in range(2)]
        act2 = ap_pool.tile([128, BPC * 196], DT_MM)
        out3 = ap_pool.tile([64, BPC * 36], DT_MM)
        def fill_imt(pp):
            nc.sync.dma_start(out=imts[pp][9:10, :], in_=ONESR[:, :G * 2304])
            for c in range(4):
                nc.sync.dma_start(
                    out=imts[pp][10:65, 4608 * c:4608 * (c + 1)],
                    in_=Z[:, :4608].to_broadcast((55, 4608)))

        def fill_act1(pp):
            for h in range(2):
                for c in range(2):
                    nc.sync.dma_start(
                        out=act1[pp][h][:, 2704 * c:2704 * (c + 1)],
                        in_=Z[:, :2704].to_broadcast((128, 2704)))

        # one-time fills, chunked to ~0.5MB and ordered by first use so
        # they never gate the early pipeline
        fill_imt(0)
        fill_act1(0)
        fill_imt(1)
        fill_act1(1)
        for c in range(4):
            nc.sync.dma_start(
                out=act2[:, 3136 * c:3136 * (c + 1)],
                in_=Z[:, :3136].to_broadcast((128, 3136)))

        with contextlib.ExitStack() as cctx:
            ps1 = cctx.enter_context(tc.tile_pool(name="ps1", bufs=1, space="PSUM"))
            ps2 = cctx.enter_context(tc.tile_pool(name="ps2", bufs=2, space="PSUM"))

            # PE warm-up: ~2.5us of dummy matmuls as soon as the weights
            # land, so the HAM clock gate reaches 8/8 before real work
            # (cold MMs run at 1.2 GHz instead of 2.4).
            psW = ps1.tile([128, 1536], F32, tag="psE", name="psW")
            for i in range(12):
                nc.tensor.matmul(out=psW[:, 0:512], lhsT=r(w1t[0:65, 0:128]),
                                 rhs=r(w2t[0:65, 0:512]), start=True, stop=True)

            def conv1_h(g, ci, imt, h):
                """conv1 half h of one image: x-parity matmuls + pooled/
                relu'd write to act1."""
                iv = imt.rearrange("p (qq b y x) -> p qq b y x",
                                   qq=2, y=48, x=24)
                if True:
                    # psum layout: 3 banks x (16 y-rows x 24 x-cols = 384)
                    psE = ps1.tile([128, 1536], F32, tag="psE", name="psE")
                    psO = ps1.tile([128, 1536], F32, tag="psO", name="psO")
                    for bk in range(3):
                        for q, ps in ((0, psE), (1, psO)):
                            nc.tensor.matmul(
                                out=ps[:, 512 * bk:512 * bk + 384],
                                lhsT=r(w1t[0:65, 128 * h:128 * (h + 1)]),
                                rhs=r(iv[0:65, q, ci,
                                         16 * bk:16 * bk + 16, :]),
                                start=True, stop=True)
                    psEv = psE[:].rearrange("p (k c) -> p k c", c=512)[:, :, 0:384]
                    psOv = psO[:].rearrange("p (k c) -> p k c", c=512)[:, :, 0:384]
                    # odd parity: relu-evac (bias already in psum via ones-row)
                    oddr = t1p.tile([128, 1152], DT_MM, tag="oddr", name="oddr")
                    oddv = oddr[:].rearrange("p (k c) -> p k c", c=384)
                    nc.scalar.activation(out=oddv, in_=psOv, func=RELU)
                    # max(even, relu(odd)) == relu(max(even, odd)) elementwise
                    m1 = t1p.tile([128, 1152], DT_MM, tag="m1", name="m1")
                    m1v3 = m1[:].rearrange("p (k c) -> p k c", c=384)
                    nc.vector.tensor_max(m1v3, psEv, oddv)
                    # y-pair max -> act1 padded interior (image ci)
                    m1v = m1[:].rearrange("p (y x) -> p y x", x=24)
                    dst = act1[g % 2][h][:].rearrange(
                        "p (b y x) -> p b y x", y=26, x=26)[:, ci, 1:25, 1:25]
                    eng = nc.gpsimd if GPS_YMAX else nc.vector
                    eng.tensor_max(dst, m1v[:, 0:48:2, :], m1v[:, 1:48:2, :])

            def conv2_rr(g, bb, rr):
                """conv2 for image bb of group g, output row-half rr."""
                a1v = [act1[g % 2][h][:].rearrange(
                    "p (b y x) -> p b y x", y=26, x=26) for h in range(2)]
                a2v = act2[:].rearrange("p (b y x) -> p b y x", y=14, x=14)
                if True:
                    ps = ps2.tile([128, 288], F32, tag="ps2")
                    i = 0
                    for t in range(9):
                        dy, dx = divmod(t, 3)
                        for h in range(2):
                            nc.tensor.matmul(
                                out=ps[:],
                                lhsT=r(w2t[:, (t * 2 + h) * 128:(t * 2 + h + 1) * 128]),
                                rhs=r(a1v[h][:, bb, 12 * rr + dy:12 * rr + dy + 12,
                                             dx:dx + 24]),
                                start=(i == 0), stop=(i == 17))
                            i += 1
                    psv = ps[:].rearrange("p (y x) -> p y x", x=24)
                    todd2 = e2p.tile([128, 144], DT_MM, tag="todd2")
                    todd2v = todd2[:].rearrange("p (y x) -> p y x", x=12)
                    nc.scalar.activation(out=todd2v, in_=psv[:, :, 1:24:2],
                                         func=RELU, bias=b2t[:])
                    t1c = e2p.tile([128, 144], DT_MM, tag="t1c2")
                    t1cv = t1c[:].rearrange("p (y x) -> p y x", x=12)
                    nc.vector.scalar_tensor_tensor(
                        out=t1cv, in0=psv[:, :, 0:24:2], scalar=b2t[:],
                        in1=todd2v, op0=ADD, op1=MAX)
                    dst = a2v[:, g * G + bb, 6 * rr + 1:6 * rr + 7, 1:13]
                    nc.vector.tensor_max(dst, t1cv[:, 0:12:2, :], t1cv[:, 1:12:2, :])

            # ---- group loop: conv1(g) interleaved with conv2(g-1) ----
            prev_g = None
            for g in range(NG):
                imt = imts[g % 2]
                if g != 0:
                    load_xpq(g)
                    load_taps(g, imt[:])
                for ci in range(G):
                    # fine interleave: each conv2 chain fills the PE while
                    # the preceding conv1 half's psum drains through the
                    # pool engines
                    for h in range(2):
                        if prev_g is not None:
                            conv2_rr(prev_g, ci, h)
                        conv1_h(g, ci, imt[:], h)
                prev_g = g
            for ci in range(G):
                for rr in range(2):
                    conv2_rr(prev_g, ci, rr)

        # ---- conv3 (all groups done; act2 complete) ----
        with contextlib.ExitStack() as cctx:
            ps3 = cctx.enter_context(tc.tile_pool(name="ps3", bufs=3, space="PSUM"))
            psq = cctx.enter_context(tc.tile_pool(name="psq", bufs=1, space="PSUM"))
            pssc = cctx.enter_context(tc.tile_pool(name="pssc", bufs=1, space="PSUM"))
            psT = cctx.enter_context(tc.tile_pool(name="psT", bufs=1, space="PSUM"))

            a2v = act2[:].rearrange("p (b y x) -> p b y x", y=14, x=14)
            o3v = out3[:].rearrange("p (b hw) -> p b hw", hw=36)
            for t in range(32):  # image pairs
                ps = ps3.tile([64, 288], F32, tag="ps3")
                for k in range(9):
                    dy, dx = divmod(k, 3)
                    nc.tensor.matmul(
                        out=ps[:],
                        lhsT=r(w3t[:, 64 * k:64 * (k + 1)]),
                        rhs=r(a2v[:, 2 * t:2 * t + 2, dy:dy + 12, dx:dx + 12]),
                        start=(k == 0), stop=(k == 8))
                psv = ps[:].rearrange("p (b y x) -> p b y x", y=12, x=12)
                todd3 = e3p.tile([64, 144], DT_MM, tag="todd3")
                todd3v = todd3[:].rearrange("p (b y x) -> p b y x", y=12, x=6)
                nc.scalar.activation(out=todd3v, in_=psv[:, :, :, 1:12:2],
                                     func=RELU, bias=b3t[:])
                t1c = e3p.tile([64, 144], DT_MM, tag="t1c3")
                t1cv = t1c[:].rearrange("p (b y x) -> p b y x", y=12, x=6)
                nc.vector.scalar_tensor_tensor(
                    out=t1cv, in0=psv[:, :, :, 0:12:2], scalar=b3t[:],
                    in1=todd3v, op0=ADD, op1=MAX)
                nc.vector.tensor_max(
                    o3v[:, 2 * t:2 * t + 2, :].rearrange("p b (y x) -> p b y x", x=6),
                    t1cv[:, :, 0:12:2, :], t1cv[:, :, 1:12:2, :])

            # ---- q = W_eff @ out4 + b_eff : accumulate over hw ----
            psq_t = psq.tile([64, 64], F32)
            for hw in range(36):
                nc.tensor.matmul(
                    out=psq_t[:],
                    lhsT=r(wet[:, 64 * hw:64 * (hw + 1)]),
                    rhs=r(out3[:, hw:2304:36]),
                    start=(hw == 0), stop=(hw == 35))
            q_sb = ap_pool.tile([64, 64], DT_MM)
            nc.vector.tensor_scalar_add(q_sb[:], psq_t[:], bet[:])

            # ---- scores: per-image matmuls -> [36, 64] psum ----
            pssc_t = pssc.tile([36, 64], F32)
            for b in range(BPC):
                nc.tensor.matmul(
                    out=pssc_t[:, b:b + 1],
                    lhsT=out3[:, 36 * b:36 * (b + 1)],
                    rhs=q_sb[:, b:b + 1],
                    start=True, stop=True)
            sc_sb = ap_pool.tile([36, 64], F32)
            nc.vector.tensor_copy(sc_sb[:], pssc_t[:])
            psT_t = psT.tile([64, 36], F32)
            nc.tensor.transpose(psT_t[:], sc_sb[:], ident[:])

            # ---- softmax over hw (free dim) ----
            mx = ap_pool.tile([64, 1], F32)
            nc.vector.tensor_reduce(out=mx[:], in_=psT_t[:],
                                    op=mybir.AluOpType.max,
                                    axis=mybir.AxisListType.X)
            nmx = ap_pool.tile([64, 1], F32)
            nc.vector.tensor_scalar_mul(nmx[:], mx[:], -1.0)
            e_t = ap_pool.tile([64, 36], F32)
            nc.scalar.activation(out=e_t[:], in_=psT_t[:], func=EXP, bias=nmx[:])
            z = ap_pool.tile([64, 1], F32)
            nc.vector.tensor_reduce(out=z[:], in_=e_t[:],
                                    op=mybir.AluOpType.add,
                                    axis=mybir.AxisListType.X)
            rz = ap_pool.tile([64, 1], F32)
            nc.vector.reciprocal(rz[:], z[:])
            attn = ap_pool.tile([64, 36], DT_MM)
            nc.vector.tensor_scalar_mul(attn[:], e_t[:], rz[:])
            if debug:
                nc.gpsimd.dma_start(out=dbg["dbg_act1_0"][:], in_=act1[0][0][:])
                nc.gpsimd.dma_start(out=dbg["dbg_act1_1"][:], in_=act1[0][1][:])
                nc.gpsimd.dma_start(out=dbg["dbg_act2"][:], in_=act2[:])
                nc.gpsimd.dma_start(out=dbg["dbg_out3"][:], in_=out3[:])
                nc.gpsimd.dma_start(out=dbg["dbg_q"][:], in_=q_sb[:])
                nc.gpsimd.dma_start(out=dbg["dbg_attn"][:], in_=attn[:])
                nc.sync.dma_start(out=dbg["dbg_sc"][:], in_=sc_sb[:])

        # ---- g_mod + fc3 ----
        with contextlib.ExitStack() as cctx:
            psab = cctx.enter_context(tc.tile_pool(name="psab", bufs=1, space="PSUM"))
            psf = cctx.enter_context(tc.tile_pool(name="psf", bufs=1, space="PSUM"))

            attn_flat = ap_pool.tile([1, 2304], DT_MM)
            nc.sync.dma_start(out=attn_flat[:], in_=attn[:])
            psab_t = psab.tile([64, 2304], F32)
            for c in range(5):
                lo = 512 * c
                hi = min(lo + 512, 2304)
                nc.tensor.matmul(out=psab_t[:, lo:hi], lhsT=r(ones1[:]),
                                 rhs=r(attn_flat[:, lo:hi]), start=True, stop=True)
            # in-place: out3 is not needed after this product
            nc.vector.tensor_mul(out3[:], out3[:], psab_t[:])
            gT = ap_pool.tile([64, 64], DT_MM)
            with nc.allow_low_precision(reason="bf16 output of attn-weighted sum"):
                nc.vector.tensor_reduce(
                    out=gT[:], in_=out3[:].rearrange("p (b hw) -> p b hw", hw=36),
                    op=mybir.AluOpType.add, axis=mybir.AxisListType.X)

            if debug:
                nc.gpsimd.dma_start(out=dbg["dbg_gT"][:], in_=gT[:])
            psf_t = psf.tile([64, 7], F32)
            nc.tensor.matmul(out=psf_t[:], lhsT=gT[:],
                             rhs=w3f[:], start=True, stop=True)
            out_sb = ap_pool.tile([64, 7], F32)
            nc.vector.tensor_add(out_sb[:], psf_t[:], fc3b_t[:])
            nc.sync.dma_start(out=out[:], in_=out_sb[:])

    _split_excess_waits(nc)
    return nc


def kernel(**inputs):
    from concourse.bass_utils import run_bass_kernel_spmd

    w = _prep_weights({k: np.asarray(v, np.float32) for k, v in inputs.items()
                       if k != 'x'})
    npdt = mybir.dt.np(DT_MM)
    for k in ('W1T', 'W2T', 'W3T', 'WeT2', 'W3fT', 'Z', 'ONESR'):
        w[k] = w[k].astype(npdt)
    xs = _prep_x(inputs['x']).astype(npdt)

    nc = build_program()
    in_maps = []
    for c in range(N_CORES):
        m = {'x': np.ascontiguousarray(xs[BPC * c:BPC * (c + 1)])}
        m.update({k: v for k, v in w.items()})
        in_maps.append(m)
    res = run_bass_kernel_spmd(nc, in_maps, list(range(N_CORES)))
    outs = [res.results[c]['out'] for c in range(N_CORES)]
    return np.concatenate(outs, axis=0).astype(np.float32)


if __name__ == '__main__':
    rng = np.random.default_rng(0)
    fake = {
        'x': rng.standard_normal((512, 1, 48, 48), dtype=np.float32),
        'conv1_w': rng.standard_normal((256, 1, 3, 3), dtype=np.float32) * 0.05,
        'conv1_b': np.zeros(256, np.float32),
        'bn1_g': np.ones(256, np.float32), 'bn1_b': np.zeros(256, np.float32),
        'bn1_m': np.zeros(256, np.float32), 'bn1_v': np.ones(256, np.float32),
        'conv2_w': rng.standard_normal((128, 256, 3, 3), dtype=np.float32) * 0.05,
        'conv2_b': np.zeros(128, np.float32),
        'bn2_g': np.ones(128, np.float32), 'bn2_b': np.zeros(128, np.float32),
        'bn2_m': np.zeros(128, np.float32), 'bn2_v': np.ones(128, np.float32),
        'conv3_w': rng.standard_normal((64, 128, 3, 3), dtype=np.float32) * 0.05,
        'conv3_b': np.zeros(64, np.float32),
        'bn3_g': np.ones(64, np.float32), 'bn3_b': np.zeros(64, np.float32),
        'bn3_m': np.zeros(64, np.float32), 'bn3_v': np.ones(64, np.float32),
        'fc1_w': rng.standard_normal((512, 2304), dtype=np.float32) * 0.05,
        'fc1_b': np.zeros(512, np.float32),
        'fc2_w': rng.standard_normal((256, 512), dtype=np.float32) * 0.05,
        'fc2_b': np.zeros(256, np.float32),
        'att_w': rng.standard_normal((64, 256), dtype=np.float32) * 0.05,
        'att_b': np.zeros(64, np.float32),
        'fc3_w': rng.standard_normal((7, 64), dtype=np.float32) * 0.05,
        'fc3_b': np.zeros(7, np.float32),
    }
    print(kernel(**fake).shape)


# revision 26
# speedup vs baseline: 1.2585x; 1.0582x over previous
"""Trainium2 Bass kernel for nn_BaselineModel_80796924772520 (dense_cnn).

Self-contained: kernel(**inputs) -> np.ndarray [512, 7] float32.

Strategy: pure data parallelism over 8 NeuronCores (64 images each).
 - BN folded into conv weights/biases on host (eval-mode BN is affine).
 - fc1/fc2/att collapse into one linear map W_eff [64, 2304] on host
   (reference has no nonlinearity between them).
 - conv1 (C_in=1, K=9): x-parity decomposition on two PE row-quadrants.
   Quadrant q in {0,1} computes the even-x / odd-x conv outputs
   concurrently (32-row PE tiling), contracting K=10 rows: 9 im2col
   taps + a ones-row that adds the folded bias inside the matmul.
   Maxpool becomes max(even, odd) in x (one DVE op against the
   ACT-relu-evacuated odd parity) then a strided y-pair max; relu is
   folded into the max tree via max(a, relu(b)) == relu(max(a, b)).
 - conv2/conv3: 9-tap shifted-window accumulating matmuls over
   zero-padded SBUF activations; pooling uses an ACT relu+bias
   evacuation of the odd-x columns, a DVE scalar_tensor_tensor
   (even + bias) max odd, and a DVE y-pair max that writes the padded
   activation (or out3) directly - no separate bias/relu pass.
 - attention: per-image [64x36]^T@[64x1] matmuls -> PE transpose ->
   softmax -> broadcast-matmul with ones -> multiply+segmented reduce.
"""
import sys
if '/opt/trn_rl_repo' not in sys.path:
    sys.path.insert(0, '/opt/trn_rl_repo')

import contextlib
import numpy as np

import concourse.bass as bass
import concourse.mybir as mybir
import concourse.tile as tile

F32 = mybir.dt.float32
BF16 = mybir.dt.bfloat16
DT_MM = BF16
RELU = mybir.ActivationFunctionType.Relu
EXP = mybir.ActivationFunctionType.Exp
ADD = mybir.AluOpType.add
MAX = mybir.AluOpType.max

N_CORES = 8
B_TOTAL = 512
BPC = B_TOTAL // N_CORES   # 64 images per core
G = 8                      # images per group
NG = BPC // G              # 8 groups
EPS = 1e-5
GPS_YMAX = False           # offload conv1 y-max to gpsimd

_MAX_WAITS = 1  # this walrus build supports 1 sync-wait per instruction


def _install_tile_fixups():
    """The nix walrus here allows only ONE sync-wait per instruction; Tile's
    exit drain aggregates one wait per live proc onto a single Drain. Spread
    the waits across spare SP nops emitted just before the drain."""
    if getattr(tile.TileContext, '_drain_patched', False):
        return

    def _patched(self, tick_clock, wait_clock):
        from concourse.vector_clock import ScopedClock
        nc = self.nc
        nops = [nc.sync.nop().ins for _ in range(32)]
        drain_inst = nc.sync.drain()
        wait_clock.add_sem_waits(
            drain_inst.ins, ScopedClock({None: tick_clock.global_clock}))
        si = drain_inst.ins.sync_info
        if si is not None and len(si.on_wait) > _MAX_WAITS:
            waits = list(si.on_wait)
            drain_inst.ins.sync_info = mybir.SyncInfo(
                on_wait=waits[:_MAX_WAITS], on_update=list(si.on_update))
            rest = waits[_MAX_WAITS:]
            for i in range(0, len(rest), _MAX_WAITS):
                nops[i // _MAX_WAITS].sync_info = mybir.SyncInfo(
                    on_wait=rest[i:i + _MAX_WAITS], on_update=[])
        nc.all_engine_barrier()
        popped = nc._tile_sem_poison_stack.pop()
        assert popped is self._sem_poison
        nc.clear_and_free_semaphores(list(self.sems.allocated().values()))
        nc.all_engine_barrier()

    tile.TileContext._drain_and_barrier = _patched
    tile.TileContext._drain_patched = True


def _split_excess_waits(nc):
    """This walrus allows one sync-wait per instruction. Hoist excess waits
    onto same-engine nops inserted immediately before the instruction
    (sequential waits on one engine are equivalent to a combined wait)."""
    idx = 0
    for f in nc.m.functions:
        for b in f.blocks:
            out, changed = [], False
            for ins in b.instructions:
                si = ins.sync_info
                if si is not None and len(si.on_wait) > _MAX_WAITS:
                    waits = list(si.on_wait)
                    extra, keep = waits[:-_MAX_WAITS], waits[-_MAX_WAITS:]
                    for j in range(0, len(extra), _MAX_WAITS):
                        nop = mybir.InstNoOp(name=f"I-wsplit-{idx}")
                        idx += 1
                        nop.engine = ins.engine
                        nop.sync_info = mybir.SyncInfo(
                            on_wait=extra[j:j + _MAX_WAITS], on_update=[])
                        nc.register_instruction(nop, overwrite=True)
                        out.append(nop)
                    ins.sync_info = mybir.SyncInfo(
                        on_wait=keep, on_update=list(si.on_update))
                    changed = True
                out.append(ins)
            if changed:
                b.instructions = out


def _prep_weights(p):
    """Fold BN, collapse FC chain, lay out weights for the device program."""
    def fold(w, b, g, be, m, v):
        inv = (g / np.sqrt(v + EPS)).astype(np.float32)
        wf = (w * inv[:, None, None, None]).astype(np.float32)
        bf = ((b - m) * inv + be).astype(np.float32)
        return wf, bf

    w1, b1 = fold(p['conv1_w'], p['conv1_b'], p['bn1_g'], p['bn1_b'], p['bn1_m'], p['bn1_v'])
    w2, b2 = fold(p['conv2_w'], p['conv2_b'], p['bn2_g'], p['bn2_b'], p['bn2_m'], p['bn2_v'])
    w3, b3 = fold(p['conv3_w'], p['conv3_b'], p['bn3_g'], p['bn3_b'], p['bn3_m'], p['bn3_v'])

    # conv1 lhsT [128, 256]: rows 32q+k (k = 3*ky+kx) = w1[c, 0, ky, kx] for
    # q in {0,1} (even-x / odd-x PE quadrants); row 32q+9 = folded bias
    # (contracted against a ones-row in the im2col buffer).
    W1T = np.zeros((128, 256), np.float32)
    W1T[0:9, :] = w1.reshape(256, 9).T
    W1T[9, :] = b1
    # conv2 lhsT [128, 2304]: [p, t*256 + h*128 + m] = w2[m, 128h+p, t]
    W2T = np.ascontiguousarray(
        w2.reshape(128, 2, 128, 9).transpose(2, 3, 1, 0)  # [p, t, h, m]
    ).reshape(128, 2304)
    # conv3 lhsT [128, 576]: [p, t*64 + m] = w3[m, p, t]
    W3T = np.ascontiguousarray(
        w3.reshape(64, 128, 9).transpose(1, 2, 0)).reshape(128, 576)

    # FC chain collapse: q = out4 @ W_eff.T + b_eff
    fc1w, fc2w, attw = p['fc1_w'], p['fc2_w'], p['att_w']
    W_eff = (attw @ fc2w @ fc1w).astype(np.float32)          # [64, 2304]
    b_eff = (attw @ (fc2w @ p['fc1_b'] + p['fc2_b']) + p['att_b']).astype(np.float32)
    # WeT2 [64, 2304]: [c, hw*64 + m] = W_eff[m, c*36 + hw]
    WeT2 = np.ascontiguousarray(
        W_eff.reshape(64, 64, 36).transpose(1, 2, 0)).reshape(64, 2304)

    W3fT = np.ascontiguousarray(p['fc3_w'].T).astype(np.float32)  # [64, 7]
    fc3b_rep = np.broadcast_to(p['fc3_b'], (64, 7)).astype(np.float32).copy()

    b2c = b2.reshape(128, 1).astype(np.float32)
    b3c = b3.reshape(64, 1).astype(np.float32)
    beffc = b_eff.reshape(64, 1).astype(np.float32)

    return dict(W1T=W1T, W2T=W2T, W3T=W3T, WeT2=WeT2, W3fT=W3fT,
                fc3b_rep=fc3b_rep, b2c=b2c, b3c=b3c, beffc=beffc,
                Z=np.zeros((1, 18432), np.float32),
                ONESR=np.ones((1, 18432), np.float32),
                IDENT=np.eye(36, dtype=np.float32))


def _prep_x(x):
    """Deinterleave x columns by parity: out[b, q*1152 + y*24 + x'] =
    x[b, y, 2x'+q]. Lets the device build a parity-separated im2col with
    contiguous DMA runs (DMA final dims must be stride-1)."""
    xr = np.asarray(x, np.float32).reshape(-1, 48, 48)
    return np.stack([xr[:, :, 0::2], xr[:, :, 1::2]],
                    axis=1).reshape(-1, 2304)


def build_program(debug=False):
    """Build the per-core SPMD Bass program. Returns nc."""
    _install_tile_fixups()
    nc = bass.Bass("TRN2", target_bir_lowering=False, debug=False)

    x = nc.declare_dram_parameter("x", [BPC, 2304], DT_MM, isOutput=False)
    W1T = nc.declare_dram_parameter("W1T", [128, 256], DT_MM, isOutput=False)
    W2T = nc.declare_dram_parameter("W2T", [128, 2304], DT_MM, isOutput=False)
    W3T = nc.declare_dram_parameter("W3T", [128, 576], DT_MM, isOutput=False)
    WeT2 = nc.declare_dram_parameter("WeT2", [64, 2304], DT_MM, isOutput=False)
    W3fT = nc.declare_dram_parameter("W3fT", [64, 7], DT_MM, isOutput=False)
    fc3b = nc.declare_dram_parameter("fc3b_rep", [64, 7], F32, isOutput=False)
    b2c = nc.declare_dram_parameter("b2c", [128, 1], F32, isOutput=False)
    b3c = nc.declare_dram_parameter("b3c", [64, 1], F32, isOutput=False)
    beffc = nc.declare_dram_parameter("beffc", [64, 1], F32, isOutput=False)
    Z = nc.declare_dram_parameter("Z", [1, 18432], DT_MM, isOutput=False)
    ONESR = nc.declare_dram_parameter("ONESR", [1, 18432], DT_MM, isOutput=False)
    IDENT = nc.declare_dram_parameter("IDENT", [36, 36], F32, isOutput=False)
    out = nc.declare_dram_parameter("out", [BPC, 7], F32, isOutput=True)
    dbg = {}
    if debug:
        for nm, shp in [("dbg_act1_0", [128, G * 676]), ("dbg_act1_1", [128, G * 676]),
                        ("dbg_act2", [128, BPC * 196]), ("dbg_out3", [64, BPC * 36]),
                        ("dbg_q", [64, 64]), ("dbg_attn", [64, 36]),
                        ("dbg_gT", [64, 64]), ("dbg_sc", [36, 64])]:
            dbg[nm] = nc.declare_dram_parameter(nm, shp, F32, isOutput=True)

    r = lambda ap: ap

    with tile.TileContext(nc) as tc, contextlib.ExitStack() as ctx:
        wp = ctx.enter_context(tc.tile_pool(name="weights", bufs=1))
        ap_pool = ctx.enter_context(tc.tile_pool(name="acts", bufs=1))
        cp = ctx.enter_context(tc.tile_pool(name="im2col", bufs=2))
        t1p = ctx.enter_context(tc.tile_pool(name="t1", bufs=3))
        e2p = ctx.enter_context(tc.tile_pool(name="ev2", bufs=6))
        e3p = ctx.enter_context(tc.tile_pool(name="ev3", bufs=3))

        # ---- group-0 input chain first, ahead of the bulk weight DMAs ----
        # xpq: zero-padded x staged as two x-parity planes per image:
        # cols p'*1250 + y_p*25 + x'_p with padded x_p = 2*x'_p + p'.
        # Host-deinterleaved x (see _prep_x) makes these loads contiguous.
        xpqs = [ap_pool.tile([8, 2500], DT_MM, tag=f"xpq{pp}",
                             name=f"xpq{pp}") for pp in range(2)]
        for pp in range(2):
            nc.sync.dma_start(out=xpqs[pp][:],
                              in_=Z[:, :2500].to_broadcast((8, 2500)))

        def load_xpq(g):
            xpqv = xpqs[g % 2][:].rearrange("p (pp y x) -> p pp y x", y=50, x=25)
            xg = x[G * g:G * (g + 1), :].rearrange("b (qq c) -> b qq c", qq=2)
            nc.gpsimd.dma_start(out=xpqv[0:G, 0, 1:49, 1:25], in_=xg[:, 1, :])
            nc.gpsimd.dma_start(out=xpqv[0:G, 1, 1:49, 0:24], in_=xg[:, 0, :])

        def load_taps(g, imt, b0=0, b1=G):
            """im2col taps: row k, parity-q block <- padded parity plane.
            Output-x parity q tap (dy,dx) reads plane (q+dx)%2 shifted.
            gpsimd (SWDGE): ~4x faster than HWDGE for this descriptor
            pattern (measured)."""
            xpqv = xpqs[g % 2][:].rearrange("p (pp y x) -> p pp y x", y=50, x=25)
            ivk = imt.rearrange("p (qq c) -> p qq c", qq=2)
            for q in range(2):
                for k in range(9):
                    dy, dx = divmod(k, 3)
                    nc.gpsimd.dma_start(
                        out=ivk[k:k + 1, q, 1152 * b0:1152 * b1],
                        in_=xpqv[b0:b1, (q + dx) % 2, dy:dy + 48,
                                 (q + dx) // 2:(q + dx) // 2 + 24])

        # Persistent im2col tiles (group-parity double buffer), [128,
        # 2*G*1152]: rows 0-8 hold tap k (rewritten per group), row 9 the
        # ones-row contracting the folded bias, rows 10-64 zero. K=65
        # keeps conv1 matmuls in the full 128x128 PE tile mode: mixing
        # 32-row-tiled matmuls with conv2's full-mode ones halves the PE
        # clock around every switch (measured: 288-col MMs at 240ns).
        imts = [ap_pool.tile([128, G * 2304], DT_MM, tag=f"imt{pp}",
                             name=f"imt{pp}") for pp in range(2)]
        load_xpq(0)
        # early-critical zero region via DVE memset (no DMA traffic):
        # conv1 contracts imt rows 10-64 (all-zero weights there guard
        # against only-finite garbage, so they must be initialized).
        # Engines need base_partition 0, so clear 0:65 and let the taps
        # overwrite rows 0-9 afterwards.
        nc.vector.memset(imts[0][0:65, :], 0.0)
        load_taps(0, imts[0][:], 0, 2)   # first two images: minimal ramp
        load_taps(0, imts[0][:], 2, G)

        # ---- load weights (ahead of the bulk zero-fills: the first
        # matmuls need them; multi-MB fills would delay them ~40us) ----
        w1t = wp.tile([128, 256], DT_MM)
        nc.sync.dma_start(out=w1t[:], in_=W1T[:])
        w2t = wp.tile([128, 2304], DT_MM)
        nc.sync.dma_start(out=w2t[:], in_=W2T[:])
        w3t = wp.tile([128, 576], DT_MM)
        nc.sync.dma_start(out=w3t[:], in_=W3T[:])
        wet = wp.tile([64, 2304], DT_MM)
        nc.sync.dma_start(out=wet[:], in_=WeT2[:])
        w3f = wp.tile([64, 7], DT_MM)
        nc.sync.dma_start(out=w3f[:], in_=W3fT[:])
        fc3b_t = wp.tile([64, 7], F32)
        nc.sync.dma_start(out=fc3b_t[:], in_=fc3b[:])
        b2t = wp.tile([128, 1], F32)
        nc.sync.dma_start(out=b2t[:], in_=b2c[:])
        b3t = wp.tile([64, 1], F32)
        nc.sync.dma_start(out=b3t[:], in_=b3c[:])
        bet = wp.tile([64, 1], F32)
        nc.sync.dma_start(out=bet[:], in_=beffc[:])
        ident = wp.tile([36, 36], F32)
        nc.sync.dma_start(out=ident[:], in_=IDENT[:])
        ones1 = wp.tile([1, 64], DT_MM)
        nc.sync.dma_start(out=ones1[:], in_=ONESR[:, 0:64])


        # ---- persistent activation buffers ----
        act1 = [[ap_pool.tile([128, G * 676], DT_MM, tag=f"act1_{pp}_{h}",
                              name=f"act1_{pp}_{h}") for h in range(2)]
                for pp in range(2)]
        act2 = ap_pool.tile([128, BPC * 196], DT_MM)
        out3 = ap_pool.tile([64, BPC * 36], DT_MM)
        def fill_imt(pp):
            nc.sync.dma_start(out=imts[pp][9:10, :], in_=ONESR[:, :G * 2304])
            for c in range(4):
                nc.sync.dma_start(
                    out=imts[pp][10:65, 4608 * c:4608 * (c + 1)],
                    in_=Z[:, :4608].to_broadcast((55, 4608)))

        def fill_act1(pp):
            for h in range(2):
                for c in range(2):
                    nc.sync.dma_start(
                        out=act1[pp][h][:, 2704 * c:2704 * (c + 1)],
                        in_=Z[:, :2704].to_broadcast((128, 2704)))

        # one-time fills: early-needed ones via DVE memset (above / here),
        # late-needed ones as chunked sync DMAs so nothing gates the
        # early pipeline
        for h in range(2):
            nc.vector.memset(act1[0][h][:], 0.0)
        nc.sync.dma_start(out=imts[0][9:10, :], in_=ONESR[:, :G * 2304])
        fill_imt(1)
        fill_act1(1)
        for c in range(4):
            nc.sync.dma_start(
                out=act2[:, 3136 * c:3136 * (c + 1)],
                in_=Z[:, :3136].to_broadcast((128, 3136)))

        with contextlib.ExitStack() as cctx:
            ps1 = cctx.enter_context(tc.tile_pool(name="ps1", bufs=1, space="PSUM"))
            ps2 = cctx.enter_context(tc.tile_pool(name="ps2", bufs=2, space="PSUM"))

            # PE warm-up: ~2.5us of dummy matmuls as soon as the weights
            # land, so the HAM clock gate reaches 8/8 before real work
            # (cold MMs run at 1.2 GHz instead of 2.4).
            psW = ps1.tile([128, 1536], F32, tag="psE", name="psW")
            for i in range(12):
                nc.tensor.matmul(out=psW[:, 0:512], lhsT=r(w1t[0:65, 0:128]),
                                 rhs=r(w2t[0:65, 0:512]), start=True, stop=True)

            def conv1_h(g, ci, imt, h):
                """conv1 half h of one image: x-parity matmuls + pooled/
                relu'd write to act1."""
                iv = imt.rearrange("p (qq b y x) -> p qq b y x",
                                   qq=2, y=48, x=24)
                if True:
                    # psum layout: 3 banks x (16 y-rows x 24 x-cols = 384)
                    psE = ps1.tile([128, 1536], F32, tag="psE", name="psE")
                    psO = ps1.tile([128, 1536], F32, tag="psO", name="psO")
                    for bk in range(3):
                        for q, ps in ((0, psE), (1, psO)):
                            nc.tensor.matmul(
                                out=ps[:, 512 * bk:512 * bk + 384],
                                lhsT=r(w1t[0:65, 128 * h:128 * (h + 1)]),
                                rhs=r(iv[0:65, q, ci,
                                         16 * bk:16 * bk + 16, :]),
                                start=True, stop=True)
                    psEv = psE[:].rearrange("p (k c) -> p k c", c=512)[:, :, 0:384]
                    psOv = psO[:].rearrange("p (k c) -> p k c", c=512)[:, :, 0:384]
                    # odd parity: relu-evac (bias already in psum via ones-row)
                    oddr = t1p.tile([128, 1152], DT_MM, tag="oddr", name="oddr")
                    oddv = oddr[:].rearrange("p (k c) -> p k c", c=384)
                    nc.scalar.activation(out=oddv, in_=psOv, func=RELU)
                    # max(even, relu(odd)) == relu(max(even, odd)) elementwise
                    m1 = t1p.tile([128, 1152], DT_MM, tag="m1", name="m1")
                    m1v3 = m1[:].rearrange("p (k c) -> p k c", c=384)
                    nc.vector.tensor_max(m1v3, psEv, oddv)
                    # y-pair max -> act1 padded interior (image ci)
                    m1v = m1[:].rearrange("p (y x) -> p y x", x=24)
                    dst = act1[g % 2][h][:].rearrange(
                        "p (b y x) -> p b y x", y=26, x=26)[:, ci, 1:25, 1:25]
                    eng = nc.gpsimd if GPS_YMAX else nc.vector
                    eng.tensor_max(dst, m1v[:, 0:48:2, :], m1v[:, 1:48:2, :])

            def conv2_rr(g, bb, rr):
                """conv2 for image bb of group g, output row-half rr."""
                a1v = [act1[g % 2][h][:].rearrange(
                    "p (b y x) -> p b y x", y=26, x=26) for h in range(2)]
                a2v = act2[:].rearrange("p (b y x) -> p b y x", y=14, x=14)
                if True:
                    ps = ps2.tile([128, 288], F32, tag="ps2")
                    i = 0
                    for t in range(9):
                        dy, dx = divmod(t, 3)
                        for h in range(2):
                            nc.tensor.matmul(
                                out=ps[:],
                                lhsT=r(w2t[:, (t * 2 + h) * 128:(t * 2 + h + 1) * 128]),
                                rhs=r(a1v[h][:, bb, 12 * rr + dy:12 * rr + dy + 12,
                                             dx:dx + 24]),
                                start=(i == 0), stop=(i == 17))
                            i += 1
                    psv = ps[:].rearrange("p (y x) -> p y x", x=24)
                    todd2 = e2p.tile([128, 144], DT_MM, tag="todd2")
                    todd2v = todd2[:].rearrange("p (y x) -> p y x", x=12)
                    nc.scalar.activation(out=todd2v, in_=psv[:, :, 1:24:2],
                                         func=RELU, bias=b2t[:])
                    t1c = e2p.tile([128, 144], DT_MM, tag="t1c2")
                    t1cv = t1c[:].rearrange("p (y x) -> p y x", x=12)
                    nc.vector.scalar_tensor_tensor(
                        out=t1cv, in0=psv[:, :, 0:24:2], scalar=b2t[:],
                        in1=todd2v, op0=ADD, op1=MAX)
                    dst = a2v[:, g * G + bb, 6 * rr + 1:6 * rr + 7, 1:13]
                    nc.vector.tensor_max(dst, t1cv[:, 0:12:2, :], t1cv[:, 1:12:2, :])

            # ---- group loop: conv1(g) interleaved with conv2(g-1) ----
            prev_g = None
            for g in range(NG):
                imt = imts[g % 2]
                if g != 0:
                    load_xpq(g)
                    load_taps(g, imt[:])
                for ci in range(G):
                    # fine interleave: each conv2 chain fills the PE while
                    # the preceding conv1 half's psum drains through the
                    # pool engines
                    for h in range(2):
                        if prev_g is not None:
                            conv2_rr(prev_g, ci, h)
                        conv1_h(g, ci, imt[:], h)
                prev_g = g
            for ci in range(G):
                for rr in range(2):
                    conv2_rr(prev_g, ci, rr)

        # ---- conv3 (all groups done; act2 complete) ----
        with contextlib.ExitStack() as cctx:
            ps3 = cctx.enter_context(tc.tile_pool(name="ps3", bufs=3, space="PSUM"))
            psq = cctx.enter_context(tc.tile_pool(name="psq", bufs=1, space="PSUM"))
            pssc = cctx.enter_context(tc.tile_pool(name="pssc", bufs=1, space="PSUM"))
            psT = cctx.enter_context(tc.tile_pool(name="psT", bufs=1, space="PSUM"))

            a2v = act2[:].rearrange("p (b y x) -> p b y x", y=14, x=14)
            o3v = out3[:].rearrange("p (b hw) -> p b hw", hw=36)
            for t in range(32):  # image pairs
                ps = ps3.tile([64, 288], F32, tag="ps3")
                for k in range(9):
                    dy, dx = divmod(k, 3)
                    nc.tensor.matmul(
                        out=ps[:],
                        lhsT=r(w3t[:, 64 * k:64 * (k + 1)]),
                        rhs=r(a2v[:, 2 * t:2 * t + 2, dy:dy + 12, dx:dx + 12]),
                        start=(k == 0), stop=(k == 8))
                psv = ps[:].rearrange("p (b y x) -> p b y x", y=12, x=12)
                todd3 = e3p.tile([64, 144], DT_MM, tag="todd3")
                todd3v = todd3[:].rearrange("p (b y x) -> p b y x", y=12, x=6)
                nc.scalar.activation(out=todd3v, in_=psv[:, :, :, 1:12:2],
                                     func=RELU, bias=b3t[:])
                t1c = e3p.tile([64, 144], DT_MM, tag="t1c3")
                t1cv = t1c[:].rearrange("p (b y x) -> p b y x", y=12, x=6)
                nc.vector.scalar_tensor_tensor(
                    out=t1cv, in0=psv[:, :, :, 0:12:2], scalar=b3t[:],
                    in1=todd3v, op0=ADD, op1=MAX)
                nc.vector.tensor_max(
                    o3v[:, 2 * t:2 * t + 2, :].rearrange("p b (y x) -> p b y x", x=6),
                    t1cv[:, :, 0:12:2, :], t1cv[:, :, 1:12:2, :])

            # ---- q = W_eff @ out4 + b_eff : accumulate over hw ----
            psq_t = psq.tile([64, 64], F32)
            for hw in range(36):
                nc.tensor.matmul(
                    out=psq_t[:],
                    lhsT=r(wet[:, 64 * hw:64 * (hw + 1)]),
                    rhs=r(out3[:, hw:2304:36]),
                    start=(hw == 0), stop=(hw == 35))
            q_sb = ap_pool.tile([64, 64], DT_MM)
            nc.vector.tensor_scalar_add(q_sb[:], psq_t[:], bet[:])

            # ---- scores: per-image matmuls -> [36, 64] psum ----
            pssc_t = pssc.tile([36, 64], F32)
            for b in range(BPC):
                nc.tensor.matmul(
                    out=pssc_t[:, b:b + 1],
                    lhsT=out3[:, 36 * b:36 * (b + 1)],
                    rhs=q_sb[:, b:b + 1],
                    start=True, stop=True)
            sc_sb = ap_pool.tile([36, 64], F32)
            nc.vector.tensor_copy(sc_sb[:], pssc_t[:])
            psT_t = psT.tile([64, 36], F32)
            nc.tensor.transpose(psT_t[:], sc_sb[:], ident[:])

            # ---- softmax over hw (free dim) ----
            mx = ap_pool.tile([64, 1], F32)
            nc.vector.tensor_reduce(out=mx[:], in_=psT_t[:],
                                    op=mybir.AluOpType.max,
                                    axis=mybir.AxisListType.X)
            nmx = ap_pool.tile([64, 1], F32)
            nc.vector.tensor_scalar_mul(nmx[:], mx[:], -1.0)
            e_t = ap_pool.tile([64, 36], F32)
            nc.scalar.activation(out=e_t[:], in_=psT_t[:], func=EXP, bias=nmx[:])
            z = ap_pool.tile([64, 1], F32)
            nc.vector.tensor_reduce(out=z[:], in_=e_t[:],
                                    op=mybir.AluOpType.add,
                                    axis=mybir.AxisListType.X)
            rz = ap_pool.tile([64, 1], F32)
            nc.vector.reciprocal(rz[:], z[:])
            attn = ap_pool.tile([64, 36], DT_MM)
            nc.vector.tensor_scalar_mul(attn[:], e_t[:], rz[:])
            if debug:
                nc.gpsimd.dma_start(out=dbg["dbg_act1_0"][:], in_=act1[0][0][:])
                nc.gpsimd.dma_start(out=dbg["dbg_act1_1"][:], in_=act1[0][1][:])
                nc.gpsimd.dma_start(out=dbg["dbg_act2"][:], in_=act2[:])
                nc.gpsimd.dma_start(out=dbg["dbg_out3"][:], in_=out3[:])
                nc.gpsimd.dma_start(out=dbg["dbg_q"][:], in_=q_sb[:])
                nc.gpsimd.dma_start(out=dbg["dbg_attn"][:], in_=attn[:])
                nc.sync.dma_start(out=dbg["dbg_sc"][:], in_=sc_sb[:])

        # ---- g_mod + fc3 ----
        with contextlib.ExitStack() as cctx:
            psab = cctx.enter_context(tc.tile_pool(name="psab", bufs=1, space="PSUM"))
            psf = cctx.enter_context(tc.tile_pool(name="psf", bufs=1, space="PSUM"))

            attn_flat = ap_pool.tile([1, 2304], DT_MM)
            nc.sync.dma_start(out=attn_flat[:], in_=attn[:])
            psab_t = psab.tile([64, 2304], F32)
            for c in range(5):
                lo = 512 * c
                hi = min(lo + 512, 2304)
                nc.tensor.matmul(out=psab_t[:, lo:hi], lhsT=r(ones1[:]),
                                 rhs=r(attn_flat[:, lo:hi]), start=True, stop=True)
            # in-place: out3 is not needed after this product
            nc.vector.tensor_mul(out3[:], out3[:], psab_t[:])
            gT = ap_pool.tile([64, 64], DT_MM)
            with nc.allow_low_precision(reason="bf16 output of attn-weighted sum"):
                nc.vector.tensor_reduce(
                    out=gT[:], in_=out3[:].rearrange("p (b hw) -> p b hw", hw=36),
                    op=mybir.AluOpType.add, axis=mybir.AxisListType.X)

            if debug:
                nc.gpsimd.dma_start(out=dbg["dbg_gT"][:], in_=gT[:])
            psf_t = psf.tile([64, 7], F32)
            nc.tensor.matmul(out=psf_t[:], lhsT=gT[:],
                             rhs=w3f[:], start=True, stop=True)
            out_sb = ap_pool.tile([64, 7], F32)
            nc.vector.tensor_add(out_sb[:], psf_t[:], fc3b_t[:])
            nc.sync.dma_start(out=out[:], in_=out_sb[:])

    _split_excess_waits(nc)
    return nc


def kernel(**inputs):
    from concourse.bass_utils import run_bass_kernel_spmd

    w = _prep_weights({k: np.asarray(v, np.float32) for k, v in inputs.items()
                       if k != 'x'})
    npdt = mybir.dt.np(DT_MM)
    for k in ('W1T', 'W2T', 'W3T', 'WeT2', 'W3fT', 'Z', 'ONESR'):
        w[k] = w[k].astype(npdt)
    xs = _prep_x(inputs['x']).astype(npdt)

    nc = build_program()
    in_maps = []
    for c in range(N_CORES):
        m = {'x': np.ascontiguousarray(xs[BPC * c:BPC * (c + 1)])}
        m.update({k: v for k, v in w.items()})
        in_maps.append(m)
    res = run_bass_kernel_spmd(nc, in_maps, list(range(N_CORES)))
    outs = [res.results[c]['out'] for c in range(N_CORES)]
    return np.concatenate(outs, axis=0).astype(np.float32)


if __name__ == '__main__':
    rng = np.random.default_rng(0)
    fake = {
        'x': rng.standard_normal((512, 1, 48, 48), dtype=np.float32),
        'conv1_w': rng.standard_normal((256, 1, 3, 3), dtype=np.float32) * 0.05,
        'conv1_b': np.zeros(256, np.float32),
        'bn1_g': np.ones(256, np.float32), 'bn1_b': np.zeros(256, np.float32),
        'bn1_m': np.zeros(256, np.float32), 'bn1_v': np.ones(256, np.float32),
        'conv2_w': rng.standard_normal((128, 256, 3, 3), dtype=np.float32) * 0.05,
        'conv2_b': np.zeros(128, np.float32),
        'bn2_g': np.ones(128, np.float32), 'bn2_b': np.zeros(128, np.float32),
        'bn2_m': np.zeros(128, np.float32), 'bn2_v': np.ones(128, np.float32),
        'conv3_w': rng.standard_normal((64, 128, 3, 3), dtype=np.float32) * 0.05,
        'conv3_b': np.zeros(64, np.float32),
        'bn3_g': np.ones(64, np.float32), 'bn3_b': np.zeros(64, np.float32),
        'bn3_m': np.zeros(64, np.float32), 'bn3_v': np.ones(64, np.float32),
        'fc1_w': rng.standard_normal((512, 2304), dtype=np.float32) * 0.05,
        'fc1_b': np.zeros(512, np.float32),
        'fc2_w': rng.standard_normal((256, 512), dtype=np.float32) * 0.05,
        'fc2_b': np.zeros(256, np.float32),
        'att_w': rng.standard_normal((64, 256), dtype=np.float32) * 0.05,
        'att_b': np.zeros(64, np.float32),
        'fc3_w': rng.standard_normal((7, 64), dtype=np.float32) * 0.05,
        'fc3_b': np.zeros(7, np.float32),
    }
    print(kernel(**fake).shape)


# revision 28
# speedup vs baseline: 1.3263x; 1.0538x over previous
"""Trainium2 Bass kernel for nn_BaselineModel_80796924772520 (dense_cnn).

Self-contained: kernel(**inputs) -> np.ndarray [512, 7] float32.

Strategy: pure data parallelism over 8 NeuronCores (64 images each).
 - BN folded into conv weights/biases on host (eval-mode BN is affine).
 - fc1/fc2/att collapse into one linear map W_eff [64, 2304] on host
   (reference has no nonlinearity between them).
 - conv1 (C_in=1, K=9): x-parity decomposition on two PE row-quadrants.
   Quadrant q in {0,1} computes the even-x / odd-x conv outputs
   concurrently (32-row PE tiling), contracting K=10 rows: 9 im2col
   taps + a ones-row that adds the folded bias inside the matmul.
   Maxpool becomes max(even, odd) in x (one DVE op against the
   ACT-relu-evacuated odd parity) then a strided y-pair max; relu is
   folded into the max tree via max(a, relu(b)) == relu(max(a, b)).
 - conv2/conv3: 9-tap shifted-window accumulating matmuls over
   zero-padded SBUF activations; pooling uses an ACT relu+bias
   evacuation of the odd-x columns, a DVE scalar_tensor_tensor
   (even + bias) max odd, and a DVE y-pair max that writes the padded
   activation (or out3) directly - no separate bias/relu pass.
 - attention: per-image [64x36]^T@[64x1] matmuls -> PE transpose ->
   softmax -> broadcast-matmul with ones -> multiply+segmented reduce.
"""
import sys
if '/opt/trn_rl_repo' not in sys.path:
    sys.path.insert(0, '/opt/trn_rl_repo')

import contextlib
import numpy as np

import concourse.bass as bass
import concourse.mybir as mybir
import concourse.tile as tile

F32 = mybir.dt.float32
BF16 = mybir.dt.bfloat16
DT_MM = BF16
RELU = mybir.ActivationFunctionType.Relu
EXP = mybir.ActivationFunctionType.Exp
ADD = mybir.AluOpType.add
MAX = mybir.AluOpType.max

N_CORES = 8
B_TOTAL = 512
BPC = B_TOTAL // N_CORES   # 64 images per core
G = 8                      # images per group
NG = BPC // G              # 8 groups
EPS = 1e-5
GPS_YMAX = False           # offload conv1 y-max to gpsimd

_MAX_WAITS = 1  # this walrus build supports 1 sync-wait per instruction


def _install_tile_fixups():
    """The nix walrus here allows only ONE sync-wait per instruction; Tile's
    exit drain aggregates one wait per live proc onto a single Drain. Spread
    the waits across spare SP nops emitted just before the drain."""
    if getattr(tile.TileContext, '_drain_patched', False):
        return

    def _patched(self, tick_clock, wait_clock):
        from concourse.vector_clock import ScopedClock
        nc = self.nc
        nops = [nc.sync.nop().ins for _ in range(32)]
        drain_inst = nc.sync.drain()
        wait_clock.add_sem_waits(
            drain_inst.ins, ScopedClock({None: tick_clock.global_clock}))
        si = drain_inst.ins.sync_info
        if si is not None and len(si.on_wait) > _MAX_WAITS:
            waits = list(si.on_wait)
            drain_inst.ins.sync_info = mybir.SyncInfo(
                on_wait=waits[:_MAX_WAITS], on_update=list(si.on_update))
            rest = waits[_MAX_WAITS:]
            for i in range(0, len(rest), _MAX_WAITS):
                nops[i // _MAX_WAITS].sync_info = mybir.SyncInfo(
                    on_wait=rest[i:i + _MAX_WAITS], on_update=[])
        nc.all_engine_barrier()
        popped = nc._tile_sem_poison_stack.pop()
        assert popped is self._sem_poison
        nc.clear_and_free_semaphores(list(self.sems.allocated().values()))
        nc.all_engine_barrier()

    tile.TileContext._drain_and_barrier = _patched
    tile.TileContext._drain_patched = True


def _split_excess_waits(nc):
    """This walrus allows one sync-wait per instruction. Hoist excess waits
    onto same-engine nops inserted immediately before the instruction
    (sequential waits on one engine are equivalent to a combined wait)."""
    idx = 0
    for f in nc.m.functions:
        for b in f.blocks:
            out, changed = [], False
            for ins in b.instructions:
                si = ins.sync_info
                if si is not None and len(si.on_wait) > _MAX_WAITS:
                    waits = list(si.on_wait)
                    extra, keep = waits[:-_MAX_WAITS], waits[-_MAX_WAITS:]
                    for j in range(0, len(extra), _MAX_WAITS):
                        nop = mybir.InstNoOp(name=f"I-wsplit-{idx}")
                        idx += 1
                        nop.engine = ins.engine
                        nop.sync_info = mybir.SyncInfo(
                            on_wait=extra[j:j + _MAX_WAITS], on_update=[])
                        nc.register_instruction(nop, overwrite=True)
                        out.append(nop)
                    ins.sync_info = mybir.SyncInfo(
                        on_wait=keep, on_update=list(si.on_update))
                    changed = True
                out.append(ins)
            if changed:
                b.instructions = out


def _prep_weights(p):
    """Fold BN, collapse FC chain, lay out weights for the device program."""
    def fold(w, b, g, be, m, v):
        inv = (g / np.sqrt(v + EPS)).astype(np.float32)
        wf = (w * inv[:, None, None, None]).astype(np.float32)
        bf = ((b - m) * inv + be).astype(np.float32)
        return wf, bf

    w1, b1 = fold(p['conv1_w'], p['conv1_b'], p['bn1_g'], p['bn1_b'], p['bn1_m'], p['bn1_v'])
    w2, b2 = fold(p['conv2_w'], p['conv2_b'], p['bn2_g'], p['bn2_b'], p['bn2_m'], p['bn2_v'])
    w3, b3 = fold(p['conv3_w'], p['conv3_b'], p['bn3_g'], p['bn3_b'], p['bn3_m'], p['bn3_v'])

    # conv1 lhsT [128, 256]: rows 32q+k (k = 3*ky+kx) = w1[c, 0, ky, kx] for
    # q in {0,1} (even-x / odd-x PE quadrants); row 32q+9 = folded bias
    # (contracted against a ones-row in the im2col buffer).
    W1T = np.zeros((128, 256), np.float32)
    W1T[0:9, :] = w1.reshape(256, 9).T
    W1T[9, :] = b1
    # conv2 lhsT [128, 2304]: [p, t*256 + h*128 + m] = w2[m, 128h+p, t]
    W2T = np.ascontiguousarray(
        w2.reshape(128, 2, 128, 9).transpose(2, 3, 1, 0)  # [p, t, h, m]
    ).reshape(128, 2304)
    # conv3 lhsT [128, 576]: [p, t*64 + m] = w3[m, p, t]
    W3T = np.ascontiguousarray(
        w3.reshape(64, 128, 9).transpose(1, 2, 0)).reshape(128, 576)

    # FC chain collapse: q = out4 @ W_eff.T + b_eff
    fc1w, fc2w, attw = p['fc1_w'], p['fc2_w'], p['att_w']
    W_eff = (attw @ fc2w @ fc1w).astype(np.float32)          # [64, 2304]
    b_eff = (attw @ (fc2w @ p['fc1_b'] + p['fc2_b']) + p['att_b']).astype(np.float32)
    # WeT2 [64, 2304]: [c, hw*64 + m] = W_eff[m, c*36 + hw]
    WeT2 = np.ascontiguousarray(
        W_eff.reshape(64, 64, 36).transpose(1, 2, 0)).reshape(64, 2304)

    W3fT = np.ascontiguousarray(p['fc3_w'].T).astype(np.float32)  # [64, 7]
    fc3b_rep = np.broadcast_to(p['fc3_b'], (64, 7)).astype(np.float32).copy()

    b2c = b2.reshape(128, 1).astype(np.float32)
    b3c = b3.reshape(64, 1).astype(np.float32)
    beffc = b_eff.reshape(64, 1).astype(np.float32)

    return dict(W1T=W1T, W2T=W2T, W3T=W3T, WeT2=WeT2, W3fT=W3fT,
                fc3b_rep=fc3b_rep, b2c=b2c, b3c=b3c, beffc=beffc,
                Z=np.zeros((1, 19200), np.float32),
                ONESR=np.ones((1, 19200), np.float32),
                IDENT=np.eye(36, dtype=np.float32))


def _prep_x(x):
    """Zero-padded x deinterleaved into parity planes: plane p' holds
    padded columns x_p = 2*x'_p + p' as [50 y_p, 25 x'_p]; 2 pad cols at
    the end so shifted flat 1200-element im2col windows stay in-bounds.
    Host-side so every device DMA is fully contiguous."""
    xr = np.asarray(x, np.float32).reshape(-1, 48, 48)
    B = xr.shape[0]
    out = np.zeros((B, 2, 50, 25), np.float32)
    out[:, 0, 1:49, 1:25] = xr[:, :, 1::2]   # odd image cols -> even padded
    out[:, 1, 1:49, 0:24] = xr[:, :, 0::2]   # even image cols -> odd padded
    return np.concatenate([out.reshape(B, 2500),
                           np.zeros((B, 2), np.float32)], axis=1)


def build_program(debug=False):
    """Build the per-core SPMD Bass program. Returns nc."""
    _install_tile_fixups()
    nc = bass.Bass("TRN2", target_bir_lowering=False, debug=False)

    x = nc.declare_dram_parameter("x", [BPC, 2502], DT_MM, isOutput=False)
    W1T = nc.declare_dram_parameter("W1T", [128, 256], DT_MM, isOutput=False)
    W2T = nc.declare_dram_parameter("W2T", [128, 2304], DT_MM, isOutput=False)
    W3T = nc.declare_dram_parameter("W3T", [128, 576], DT_MM, isOutput=False)
    WeT2 = nc.declare_dram_parameter("WeT2", [64, 2304], DT_MM, isOutput=False)
    W3fT = nc.declare_dram_parameter("W3fT", [64, 7], DT_MM, isOutput=False)
    fc3b = nc.declare_dram_parameter("fc3b_rep", [64, 7], F32, isOutput=False)
    b2c = nc.declare_dram_parameter("b2c", [128, 1], F32, isOutput=False)
    b3c = nc.declare_dram_parameter("b3c", [64, 1], F32, isOutput=False)
    beffc = nc.declare_dram_parameter("beffc", [64, 1], F32, isOutput=False)
    Z = nc.declare_dram_parameter("Z", [1, 19200], DT_MM, isOutput=False)
    ONESR = nc.declare_dram_parameter("ONESR", [1, 19200], DT_MM, isOutput=False)
    IDENT = nc.declare_dram_parameter("IDENT", [36, 36], F32, isOutput=False)
    out = nc.declare_dram_parameter("out", [BPC, 7], F32, isOutput=True)
    dbg = {}
    if debug:
        for nm, shp in [("dbg_act1_0", [128, G * 676]), ("dbg_act1_1", [128, G * 676]),
                        ("dbg_act2", [128, BPC * 196]), ("dbg_out3", [64, BPC * 36]),
                        ("dbg_q", [64, 64]), ("dbg_attn", [64, 36]),
                        ("dbg_gT", [64, 64]), ("dbg_sc", [36, 64])]:
            dbg[nm] = nc.declare_dram_parameter(nm, shp, F32, isOutput=True)

    r = lambda ap: ap

    with tile.TileContext(nc) as tc, contextlib.ExitStack() as ctx:
        wp = ctx.enter_context(tc.tile_pool(name="weights", bufs=1))
        ap_pool = ctx.enter_context(tc.tile_pool(name="acts", bufs=1))
        cp = ctx.enter_context(tc.tile_pool(name="im2col", bufs=2))
        t1p = ctx.enter_context(tc.tile_pool(name="t1", bufs=3))
        e2p = ctx.enter_context(tc.tile_pool(name="ev2", bufs=6))
        e3p = ctx.enter_context(tc.tile_pool(name="ev3", bufs=3))

        # ---- group-0 input chain first, ahead of the bulk weight DMAs ----
        # xpq: zero-padded x staged as two x-parity planes per image:
        # cols p'*1250 + y_p*25 + x'_p with padded x_p = 2*x'_p + p'.
        # Host-deinterleaved x (see _prep_x) makes these loads contiguous.
        xpqs = [ap_pool.tile([8, 2502], DT_MM, tag=f"xpq{pp}",
                             name=f"xpq{pp}") for pp in range(2)]

        def load_xpq(g):
            nc.gpsimd.dma_start(out=xpqs[g % 2][:],
                                in_=x[G * g:G * (g + 1), :])

        def load_taps(g, imt, b0=0, b1=G):
            """im2col taps: row k, parity-q block <- padded parity plane.
            Output-x parity q tap (dy,dx) reads plane (q+dx)%2 shifted.
            gpsimd (SWDGE): ~4x faster than HWDGE for this descriptor
            pattern (measured)."""
            xpq = xpqs[g % 2]
            ivk = imt.rearrange("p (qq c) -> p qq c", qq=2)
            for q in range(2):
                for k in range(9):
                    dy, dx = divmod(k, 3)
                    lo = 1250 * ((q + dx) % 2) + 25 * dy + (q + dx) // 2
                    nc.gpsimd.dma_start(
                        out=ivk[k:k + 1, q, 1200 * b0:1200 * b1],
                        in_=xpq[b0:b1, lo:lo + 1200])

        # Persistent im2col tiles (group-parity double buffer), [128,
        # 2*G*1152]: rows 0-8 hold tap k (rewritten per group), row 9 the
        # ones-row contracting the folded bias, rows 10-64 zero. K=65
        # keeps conv1 matmuls in the full 128x128 PE tile mode: mixing
        # 32-row-tiled matmuls with conv2's full-mode ones halves the PE
        # clock around every switch (measured: 288-col MMs at 240ns).
        imts = [ap_pool.tile([128, G * 2400], DT_MM, tag=f"imt{pp}",
                             name=f"imt{pp}") for pp in range(2)]
        load_xpq(0)
        # early-critical zero region via DVE memset (no DMA traffic):
        # conv1 contracts imt rows 10-64 (all-zero weights there guard
        # against only-finite garbage, so they must be initialized).
        # Engines need base_partition 0, so clear 0:65 and let the taps
        # overwrite rows 0-9 afterwards.
        load_taps(0, imts[0][:])

        # ---- load weights (ahead of the bulk zero-fills: the first
        # matmuls need them; multi-MB fills would delay them ~40us) ----
        w1t = wp.tile([128, 256], DT_MM)
        nc.sync.dma_start(out=w1t[:], in_=W1T[:])
        w2t = wp.tile([128, 2304], DT_MM)
        nc.sync.dma_start(out=w2t[:], in_=W2T[:])
        w3t = wp.tile([128, 576], DT_MM)
        nc.sync.dma_start(out=w3t[:], in_=W3T[:])
        wet = wp.tile([64, 2304], DT_MM)
        nc.sync.dma_start(out=wet[:], in_=WeT2[:])
        w3f = wp.tile([64, 7], DT_MM)
        nc.sync.dma_start(out=w3f[:], in_=W3fT[:])
        fc3b_t = wp.tile([64, 7], F32)
        nc.sync.dma_start(out=fc3b_t[:], in_=fc3b[:])
        b2t = wp.tile([128, 1], F32)
        nc.sync.dma_start(out=b2t[:], in_=b2c[:])
        b3t = wp.tile([64, 1], F32)
        nc.sync.dma_start(out=b3t[:], in_=b3c[:])
        bet = wp.tile([64, 1], F32)
        nc.sync.dma_start(out=bet[:], in_=beffc[:])
        ident = wp.tile([36, 36], F32)
        nc.sync.dma_start(out=ident[:], in_=IDENT[:])
        ones1 = wp.tile([1, 64], DT_MM)
        nc.sync.dma_start(out=ones1[:], in_=ONESR[:, 0:64])


        # ---- persistent activation buffers ----
        act1 = [[ap_pool.tile([128, G * 676], DT_MM, tag=f"act1_{pp}_{h}",
                              name=f"act1_{pp}_{h}") for h in range(2)]
                for pp in range(2)]
        act2 = ap_pool.tile([128, BPC * 196], DT_MM)
        out3 = ap_pool.tile([64, BPC * 36], DT_MM)
        def fill_imt(pp):
            nc.sync.dma_start(out=imts[pp][9:10, :], in_=ONESR[:, :G * 2400])
            for c in range(4):
                nc.sync.dma_start(
                    out=imts[pp][10:65, 4800 * c:4800 * (c + 1)],
                    in_=Z[:, :4800].to_broadcast((55, 4800)))

        def fill_act1(pp):
            for h in range(2):
                for c in range(2):
                    nc.sync.dma_start(
                        out=act1[pp][h][:, 2704 * c:2704 * (c + 1)],
                        in_=Z[:, :2704].to_broadcast((128, 2704)))

        # one-time fills: early-needed ones via DVE memset, late-needed
        # ones as chunked sync DMAs so nothing gates the early pipeline
        for h in range(2):
            nc.vector.memset(act1[0][h][:], 0.0)
        fill_imt(0)
        fill_imt(1)
        fill_act1(1)
        for c in range(4):
            nc.sync.dma_start(
                out=act2[:, 3136 * c:3136 * (c + 1)],
                in_=Z[:, :3136].to_broadcast((128, 3136)))

        with contextlib.ExitStack() as cctx:
            ps1 = cctx.enter_context(tc.tile_pool(name="ps1", bufs=1, space="PSUM"))
            ps2 = cctx.enter_context(tc.tile_pool(name="ps2", bufs=2, space="PSUM"))

            # PE warm-up: ~2.5us of dummy matmuls as soon as the weights
            # land, so the HAM clock gate reaches 8/8 before real work
            # (cold MMs run at 1.2 GHz instead of 2.4).
            psW = ps1.tile([128, 1536], F32, tag="psE", name="psW")
            for i in range(12):
                nc.tensor.matmul(out=psW[:, 0:512], lhsT=r(w1t[0:65, 0:128]),
                                 rhs=r(w2t[0:65, 0:512]), start=True, stop=True)

            def conv1_h(g, ci, imt, h):
                """conv1 half h of one image: x-parity matmuls + pooled/
                relu'd write to act1."""
                iv = imt.rearrange("p (qq b y x) -> p qq b y x",
                                   qq=2, y=48, x=25)
                if True:
                    # psum layout: 3 banks x (16 y-rows x 24 x-cols = 384)
                    psE = ps1.tile([128, 1536], F32, tag="psE", name="psE")
                    psO = ps1.tile([128, 1536], F32, tag="psO", name="psO")
                    for bk in range(3):
                        for q, ps in ((0, psE), (1, psO)):
                            nc.tensor.matmul(
                                out=ps[:, 512 * bk:512 * bk + 384],
                                lhsT=r(w1t[0:65, 128 * h:128 * (h + 1)]),
                                rhs=r(iv[0:65, q, ci,
                                         16 * bk:16 * bk + 16, 0:24]),
                                start=True, stop=True)
                    psEv = psE[:].rearrange("p (k c) -> p k c", c=512)[:, :, 0:384]
                    psOv = psO[:].rearrange("p (k c) -> p k c", c=512)[:, :, 0:384]
                    # odd parity: relu-evac (bias already in psum via ones-row)
                    oddr = t1p.tile([128, 1152], DT_MM, tag="oddr", name="oddr")
                    oddv = oddr[:].rearrange("p (k c) -> p k c", c=384)
                    nc.scalar.activation(out=oddv, in_=psOv, func=RELU)
                    # max(even, relu(odd)) == relu(max(even, odd)) elementwise
                    m1 = t1p.tile([128, 1152], DT_MM, tag="m1", name="m1")
                    m1v3 = m1[:].rearrange("p (k c) -> p k c", c=384)
                    nc.vector.tensor_max(m1v3, psEv, oddv)
                    # y-pair max -> act1 padded interior (image ci)
                    m1v = m1[:].rearrange("p (y x) -> p y x", x=24)
                    dst = act1[g % 2][h][:].rearrange(
                        "p (b y x) -> p b y x", y=26, x=26)[:, ci, 1:25, 1:25]
                    eng = nc.gpsimd if GPS_YMAX else nc.vector
                    eng.tensor_max(dst, m1v[:, 0:48:2, :], m1v[:, 1:48:2, :])

            def conv2_rr(g, bb, rr):
                """conv2 for image bb of group g, output row-half rr."""
                a1v = [act1[g % 2][h][:].rearrange(
                    "p (b y x) -> p b y x", y=26, x=26) for h in range(2)]
                a2v = act2[:].rearrange("p (b y x) -> p b y x", y=14, x=14)
                if True:
                    ps = ps2.tile([128, 288], F32, tag="ps2")
                    i = 0
                    for t in range(9):
                        dy, dx = divmod(t, 3)
                        for h in range(2):
                            nc.tensor.matmul(
                                out=ps[:],
                                lhsT=r(w2t[:, (t * 2 + h) * 128:(t * 2 + h + 1) * 128]),
                                rhs=r(a1v[h][:, bb, 12 * rr + dy:12 * rr + dy + 12,
                                             dx:dx + 24]),
                                start=(i == 0), stop=(i == 17))
                            i += 1
                    psv = ps[:].rearrange("p (y x) -> p y x", x=24)
                    todd2 = e2p.tile([128, 144], DT_MM, tag="todd2")
                    todd2v = todd2[:].rearrange("p (y x) -> p y x", x=12)
                    nc.scalar.activation(out=todd2v, in_=psv[:, :, 1:24:2],
                                         func=RELU, bias=b2t[:])
                    t1c = e2p.tile([128, 144], DT_MM, tag="t1c2")
                    t1cv = t1c[:].rearrange("p (y x) -> p y x", x=12)
                    nc.vector.scalar_tensor_tensor(
                        out=t1cv, in0=psv[:, :, 0:24:2], scalar=b2t[:],
                        in1=todd2v, op0=ADD, op1=MAX)
                    dst = a2v[:, g * G + bb, 6 * rr + 1:6 * rr + 7, 1:13]
                    nc.vector.tensor_max(dst, t1cv[:, 0:12:2, :], t1cv[:, 1:12:2, :])

            # ---- group loop: conv1(g) interleaved with conv2(g-1) ----
            prev_g = None
            for g in range(NG):
                imt = imts[g % 2]
                if g != 0:
                    load_xpq(g)
                    load_taps(g, imt[:])
                for ci in range(G):
                    # fine interleave: each conv2 chain fills the PE while
                    # the preceding conv1 half's psum drains through the
                    # pool engines
                    for h in range(2):
                        if prev_g is not None:
                            conv2_rr(prev_g, ci, h)
                        conv1_h(g, ci, imt[:], h)
                prev_g = g
            for ci in range(G):
                for rr in range(2):
                    conv2_rr(prev_g, ci, rr)

        # ---- conv3 (all groups done; act2 complete) ----
        with contextlib.ExitStack() as cctx:
            ps3 = cctx.enter_context(tc.tile_pool(name="ps3", bufs=3, space="PSUM"))
            psq = cctx.enter_context(tc.tile_pool(name="psq", bufs=1, space="PSUM"))
            pssc = cctx.enter_context(tc.tile_pool(name="pssc", bufs=1, space="PSUM"))
            psT = cctx.enter_context(tc.tile_pool(name="psT", bufs=1, space="PSUM"))

            a2v = act2[:].rearrange("p (b y x) -> p b y x", y=14, x=14)
            o3v = out3[:].rearrange("p (b hw) -> p b hw", hw=36)
            for t in range(32):  # image pairs
                ps = ps3.tile([64, 288], F32, tag="ps3")
                for k in range(9):
                    dy, dx = divmod(k, 3)
                    nc.tensor.matmul(
                        out=ps[:],
                        lhsT=r(w3t[:, 64 * k:64 * (k + 1)]),
                        rhs=r(a2v[:, 2 * t:2 * t + 2, dy:dy + 12, dx:dx + 12]),
                        start=(k == 0), stop=(k == 8))
                psv = ps[:].rearrange("p (b y x) -> p b y x", y=12, x=12)
                todd3 = e3p.tile([64, 144], DT_MM, tag="todd3")
                todd3v = todd3[:].rearrange("p (b y x) -> p b y x", y=12, x=6)
                nc.scalar.activation(out=todd3v, in_=psv[:, :, :, 1:12:2],
                                     func=RELU, bias=b3t[:])
                t1c = e3p.tile([64, 144], DT_MM, tag="t1c3")
                t1cv = t1c[:].rearrange("p (b y x) -> p b y x", y=12, x=6)
                nc.vector.scalar_tensor_tensor(
                    out=t1cv, in0=psv[:, :, :, 0:12:2], scalar=b3t[:],
                    in1=todd3v, op0=ADD, op1=MAX)
                nc.vector.tensor_max(
                    o3v[:, 2 * t:2 * t + 2, :].rearrange("p b (y x) -> p b y x", x=6),
                    t1cv[:, :, 0:12:2, :], t1cv[:, :, 1:12:2, :])

            # ---- q = W_eff @ out4 + b_eff : accumulate over hw ----
            psq_t = psq.tile([64, 64], F32)
            for hw in range(36):
                nc.tensor.matmul(
                    out=psq_t[:],
                    lhsT=r(wet[:, 64 * hw:64 * (hw + 1)]),
                    rhs=r(out3[:, hw:2304:36]),
                    start=(hw == 0), stop=(hw == 35))
            q_sb = ap_pool.tile([64, 64], DT_MM)
            nc.vector.tensor_scalar_add(q_sb[:], psq_t[:], bet[:])

            # ---- scores: per-image matmuls -> [36, 64] psum ----
            pssc_t = pssc.tile([36, 64], F32)
            for b in range(BPC):
                nc.tensor.matmul(
                    out=pssc_t[:, b:b + 1],
                    lhsT=out3[:, 36 * b:36 * (b + 1)],
                    rhs=q_sb[:, b:b + 1],
                    start=True, stop=True)
            sc_sb = ap_pool.tile([36, 64], F32)
            nc.vector.tensor_copy(sc_sb[:], pssc_t[:])
            psT_t = psT.tile([64, 36], F32)
            nc.tensor.transpose(psT_t[:], sc_sb[:], ident[:])

            # ---- softmax over hw (free dim) ----
            mx = ap_pool.tile([64, 1], F32)
            nc.vector.tensor_reduce(out=mx[:], in_=psT_t[:],
                                    op=mybir.AluOpType.max,
                                    axis=mybir.AxisListType.X)
            nmx = ap_pool.tile([64, 1], F32)
            nc.vector.tensor_scalar_mul(nmx[:], mx[:], -1.0)
            e_t = ap_pool.tile([64, 36], F32)
            nc.scalar.activation(out=e_t[:], in_=psT_t[:], func=EXP, bias=nmx[:])
            z = ap_pool.tile([64, 1], F32)
            nc.vector.tensor_reduce(out=z[:], in_=e_t[:],
                                    op=mybir.AluOpType.add,
                                    axis=mybir.AxisListType.X)
            rz = ap_pool.tile([64, 1], F32)
            nc.vector.reciprocal(rz[:], z[:])
            attn = ap_pool.tile([64, 36], DT_MM)
            nc.vector.tensor_scalar_mul(attn[:], e_t[:], rz[:])
            if debug:
                nc.gpsimd.dma_start(out=dbg["dbg_act1_0"][:], in_=act1[0][0][:])
                nc.gpsimd.dma_start(out=dbg["dbg_act1_1"][:], in_=act1[0][1][:])
                nc.gpsimd.dma_start(out=dbg["dbg_act2"][:], in_=act2[:])
                nc.gpsimd.dma_start(out=dbg["dbg_out3"][:], in_=out3[:])
                nc.gpsimd.dma_start(out=dbg["dbg_q"][:], in_=q_sb[:])
                nc.gpsimd.dma_start(out=dbg["dbg_attn"][:], in_=attn[:])
                nc.sync.dma_start(out=dbg["dbg_sc"][:], in_=sc_sb[:])

        # ---- g_mod + fc3 ----
        with contextlib.ExitStack() as cctx:
            psab = cctx.enter_context(tc.tile_pool(name="psab", bufs=1, space="PSUM"))
            psf = cctx.enter_context(tc.tile_pool(name="psf", bufs=1, space="PSUM"))

            attn_flat = ap_pool.tile([1, 2304], DT_MM)
            nc.sync.dma_start(out=attn_flat[:], in_=attn[:])
            psab_t = psab.tile([64, 2304], F32)
            for c in range(5):
                lo = 512 * c
                hi = min(lo + 512, 2304)
                nc.tensor.matmul(out=psab_t[:, lo:hi], lhsT=r(ones1[:]),
                                 rhs=r(attn_flat[:, lo:hi]), start=True, stop=True)
            # in-place: out3 is not needed after this product
            nc.vector.tensor_mul(out3[:], out3[:], psab_t[:])
            gT = ap_pool.tile([64, 64], DT_MM)
            with nc.allow_low_precision(reason="bf16 output of attn-weighted sum"):
                nc.vector.tensor_reduce(
                    out=gT[:], in_=out3[:].rearrange("p (b hw) -> p b hw", hw=36),
                    op=mybir.AluOpType.add, axis=mybir.AxisListType.X)

            if debug:
                nc.gpsimd.dma_start(out=dbg["dbg_gT"][:], in_=gT[:])
            psf_t = psf.tile([64, 7], F32)
            nc.tensor.matmul(out=psf_t[:], lhsT=gT[:],
                             rhs=w3f[:], start=True, stop=True)
            out_sb = ap_pool.tile([64, 7], F32)
            nc.vector.tensor_add(out_sb[:], psf_t[:], fc3b_t[:])
            nc.sync.dma_start(out=out[:], in_=out_sb[:])

    _split_excess_waits(nc)
    return nc


def kernel(**inputs):
    from concourse.bass_utils import run_bass_kernel_spmd

    w = _prep_weights({k: np.asarray(v, np.float32) for k, v in inputs.items()
                       if k != 'x'})
    npdt = mybir.dt.np(DT_MM)
    for k in ('W1T', 'W2T', 'W3T', 'WeT2', 'W3fT', 'Z', 'ONESR'):
        w[k] = w[k].astype(npdt)
    xs = _prep_x(inputs['x']).astype(npdt)

    nc = build_program()
    in_maps = []
    for c in range(N_CORES):
        m = {'x': np.ascontiguousarray(xs[BPC * c:BPC * (c + 1)])}
        m.update({k: v for k, v in w.items()})
        in_maps.append(m)
    res = run_bass_kernel_spmd(nc, in_maps, list(range(N_CORES)))
    outs = [res.results[c]['out'] for c in range(N_CORES)]
    return np.concatenate(outs, axis=0).astype(np.float32)


if __name__ == '__main__':
    rng = np.random.default_rng(0)
    fake = {
        'x': rng.standard_normal((512, 1, 48, 48), dtype=np.float32),
        'conv1_w': rng.standard_normal((256, 1, 3, 3), dtype=np.float32) * 0.05,
        'conv1_b': np.zeros(256, np.float32),
        'bn1_g': np.ones(256, np.float32), 'bn1_b': np.zeros(256, np.float32),
        'bn1_m': np.zeros(256, np.float32), 'bn1_v': np.ones(256, np.float32),
        'conv2_w': rng.standard_normal((128, 256, 3, 3), dtype=np.float32) * 0.05,
        'conv2_b': np.zeros(128, np.float32),
        'bn2_g': np.ones(128, np.float32), 'bn2_b': np.zeros(128, np.float32),
        'bn2_m': np.zeros(128, np.float32), 'bn2_v': np.ones(128, np.float32),
        'conv3_w': rng.standard_normal((64, 128, 3, 3), dtype=np.float32) * 0.05,
        'conv3_b': np.zeros(64, np.float32),
        'bn3_g': np.ones(64, np.float32), 'bn3_b': np.zeros(64, np.float32),
        'bn3_m': np.zeros(64, np.float32), 'bn3_v': np.ones(64, np.float32),
        'fc1_w': rng.standard_normal((512, 2304), dtype=np.float32) * 0.05,
        'fc1_b': np.zeros(512, np.float32),
        'fc2_w': rng.standard_normal((256, 512), dtype=np.float32) * 0.05,
        'fc2_b': np.zeros(256, np.float32),
        'att_w': rng.standard_normal((64, 256), dtype=np.float32) * 0.05,
        'att_b': np.zeros(64, np.float32),
        'fc3_w': rng.standard_normal((7, 64), dtype=np.float32) * 0.05,
        'fc3_b': np.zeros(7, np.float32),
    }
    print(kernel(**fake).shape)


# revision 29
# speedup vs baseline: 1.3347x; 1.0064x over previous
"""Trainium2 Bass kernel for nn_BaselineModel_80796924772520 (dense_cnn).

Self-contained: kernel(**inputs) -> np.ndarray [512, 7] float32.

Strategy: pure data parallelism over 8 NeuronCores (64 images each).
 - BN folded into conv weights/biases on host (eval-mode BN is affine).
 - fc1/fc2/att collapse into one linear map W_eff [64, 2304] on host
   (reference has no nonlinearity between them).
 - conv1 (C_in=1, K=9): x-parity decomposition on two PE row-quadrants.
   Quadrant q in {0,1} computes the even-x / odd-x conv outputs
   concurrently (32-row PE tiling), contracting K=10 rows: 9 im2col
   taps + a ones-row that adds the folded bias inside the matmul.
   Maxpool becomes max(even, odd) in x (one DVE op against the
   ACT-relu-evacuated odd parity) then a strided y-pair max; relu is
   folded into the max tree via max(a, relu(b)) == relu(max(a, b)).
 - conv2/conv3: 9-tap shifted-window accumulating matmuls over
   zero-padded SBUF activations; pooling uses an ACT relu+bias
   evacuation of the odd-x columns, a DVE scalar_tensor_tensor
   (even + bias) max odd, and a DVE y-pair max that writes the padded
   activation (or out3) directly - no separate bias/relu pass.
 - attention: per-image [64x36]^T@[64x1] matmuls -> PE transpose ->
   softmax -> broadcast-matmul with ones -> multiply+segmented reduce.
"""
import sys
if '/opt/trn_rl_repo' not in sys.path:
    sys.path.insert(0, '/opt/trn_rl_repo')

import contextlib
import numpy as np

import concourse.bass as bass
import concourse.mybir as mybir
import concourse.tile as tile

F32 = mybir.dt.float32
BF16 = mybir.dt.bfloat16
DT_MM = BF16
RELU = mybir.ActivationFunctionType.Relu
EXP = mybir.ActivationFunctionType.Exp
ADD = mybir.AluOpType.add
MAX = mybir.AluOpType.max

N_CORES = 8
B_TOTAL = 512
BPC = B_TOTAL // N_CORES   # 64 images per core
G = 8                      # images per group
NG = BPC // G              # 8 groups
EPS = 1e-5
GPS_YMAX = False           # offload conv1 y-max to gpsimd

_MAX_WAITS = 1  # this walrus build supports 1 sync-wait per instruction


def _install_tile_fixups():
    """The nix walrus here allows only ONE sync-wait per instruction; Tile's
    exit drain aggregates one wait per live proc onto a single Drain. Spread
    the waits across spare SP nops emitted just before the drain."""
    if getattr(tile.TileContext, '_drain_patched', False):
        return

    def _patched(self, tick_clock, wait_clock):
        from concourse.vector_clock import ScopedClock
        nc = self.nc
        nops = [nc.sync.nop().ins for _ in range(32)]
        drain_inst = nc.sync.drain()
        wait_clock.add_sem_waits(
            drain_inst.ins, ScopedClock({None: tick_clock.global_clock}))
        si = drain_inst.ins.sync_info
        if si is not None and len(si.on_wait) > _MAX_WAITS:
            waits = list(si.on_wait)
            drain_inst.ins.sync_info = mybir.SyncInfo(
                on_wait=waits[:_MAX_WAITS], on_update=list(si.on_update))
            rest = waits[_MAX_WAITS:]
            for i in range(0, len(rest), _MAX_WAITS):
                nops[i // _MAX_WAITS].sync_info = mybir.SyncInfo(
                    on_wait=rest[i:i + _MAX_WAITS], on_update=[])
        nc.all_engine_barrier()
        popped = nc._tile_sem_poison_stack.pop()
        assert popped is self._sem_poison
        nc.clear_and_free_semaphores(list(self.sems.allocated().values()))
        nc.all_engine_barrier()

    tile.TileContext._drain_and_barrier = _patched
    tile.TileContext._drain_patched = True


def _split_excess_waits(nc):
    """This walrus allows one sync-wait per instruction. Hoist excess waits
    onto same-engine nops inserted immediately before the instruction
    (sequential waits on one engine are equivalent to a combined wait)."""
    idx = 0
    for f in nc.m.functions:
        for b in f.blocks:
            out, changed = [], False
            for ins in b.instructions:
                si = ins.sync_info
                if si is not None and len(si.on_wait) > _MAX_WAITS:
                    waits = list(si.on_wait)
                    extra, keep = waits[:-_MAX_WAITS], waits[-_MAX_WAITS:]
                    for j in range(0, len(extra), _MAX_WAITS):
                        nop = mybir.InstNoOp(name=f"I-wsplit-{idx}")
                        idx += 1
                        nop.engine = ins.engine
                        nop.sync_info = mybir.SyncInfo(
                            on_wait=extra[j:j + _MAX_WAITS], on_update=[])
                        nc.register_instruction(nop, overwrite=True)
                        out.append(nop)
                    ins.sync_info = mybir.SyncInfo(
                        on_wait=keep, on_update=list(si.on_update))
                    changed = True
                out.append(ins)
            if changed:
                b.instructions = out


def _prep_weights(p):
    """Fold BN, collapse FC chain, lay out weights for the device program."""
    def fold(w, b, g, be, m, v):
        inv = (g / np.sqrt(v + EPS)).astype(np.float32)
        wf = (w * inv[:, None, None, None]).astype(np.float32)
        bf = ((b - m) * inv + be).astype(np.float32)
        return wf, bf

    w1, b1 = fold(p['conv1_w'], p['conv1_b'], p['bn1_g'], p['bn1_b'], p['bn1_m'], p['bn1_v'])
    w2, b2 = fold(p['conv2_w'], p['conv2_b'], p['bn2_g'], p['bn2_b'], p['bn2_m'], p['bn2_v'])
    w3, b3 = fold(p['conv3_w'], p['conv3_b'], p['bn3_g'], p['bn3_b'], p['bn3_m'], p['bn3_v'])

    # conv1 lhsT [128, 256]: rows 32q+k (k = 3*ky+kx) = w1[c, 0, ky, kx] for
    # q in {0,1} (even-x / odd-x PE quadrants); row 32q+9 = folded bias
    # (contracted against a ones-row in the im2col buffer).
    W1T = np.zeros((128, 256), np.float32)
    W1T[0:9, :] = w1.reshape(256, 9).T
    W1T[9, :] = b1
    # conv2 lhsT [128, 2304]: [p, t*256 + h*128 + m] = w2[m, 128h+p, t]
    W2T = np.ascontiguousarray(
        w2.reshape(128, 2, 128, 9).transpose(2, 3, 1, 0)  # [p, t, h, m]
    ).reshape(128, 2304)
    # conv3 lhsT [128, 576]: [p, t*64 + m] = w3[m, p, t]
    W3T = np.ascontiguousarray(
        w3.reshape(64, 128, 9).transpose(1, 2, 0)).reshape(128, 576)

    # FC chain collapse: q = out4 @ W_eff.T + b_eff
    fc1w, fc2w, attw = p['fc1_w'], p['fc2_w'], p['att_w']
    W_eff = (attw @ fc2w @ fc1w).astype(np.float32)          # [64, 2304]
    b_eff = (attw @ (fc2w @ p['fc1_b'] + p['fc2_b']) + p['att_b']).astype(np.float32)
    # WeT2 [64, 2304]: [c, hw*64 + m] = W_eff[m, c*36 + hw]
    WeT2 = np.ascontiguousarray(
        W_eff.reshape(64, 64, 36).transpose(1, 2, 0)).reshape(64, 2304)

    W3fT = np.ascontiguousarray(p['fc3_w'].T).astype(np.float32)  # [64, 7]
    fc3b_rep = np.broadcast_to(p['fc3_b'], (64, 7)).astype(np.float32).copy()

    b2c = b2.reshape(128, 1).astype(np.float32)
    b3c = b3.reshape(64, 1).astype(np.float32)
    beffc = b_eff.reshape(64, 1).astype(np.float32)

    return dict(W1T=W1T, W2T=W2T, W3T=W3T, WeT2=WeT2, W3fT=W3fT,
                fc3b_rep=fc3b_rep, b2c=b2c, b3c=b3c, beffc=beffc,
                Z=np.zeros((1, 19200), np.float32),
                ONESR=np.ones((1, 19200), np.float32),
                IDENT=np.eye(36, dtype=np.float32))


def _prep_x(x):
    """Zero-padded x deinterleaved into parity planes: plane p' holds
    padded columns x_p = 2*x'_p + p' as [50 y_p, 25 x'_p]; 2 pad cols at
    the end so shifted flat 1200-element im2col windows stay in-bounds.
    Host-side so every device DMA is fully contiguous."""
    xr = np.asarray(x, np.float32).reshape(-1, 48, 48)
    B = xr.shape[0]
    out = np.zeros((B, 2, 50, 25), np.float32)
    out[:, 0, 1:49, 1:25] = xr[:, :, 1::2]   # odd image cols -> even padded
    out[:, 1, 1:49, 0:24] = xr[:, :, 0::2]   # even image cols -> odd padded
    return np.concatenate([out.reshape(B, 2500),
                           np.zeros((B, 2), np.float32)], axis=1)


def build_program(debug=False):
    """Build the per-core SPMD Bass program. Returns nc."""
    _install_tile_fixups()
    nc = bass.Bass("TRN2", target_bir_lowering=False, debug=False)

    x = nc.declare_dram_parameter("x", [BPC, 2502], DT_MM, isOutput=False)
    W1T = nc.declare_dram_parameter("W1T", [128, 256], DT_MM, isOutput=False)
    W2T = nc.declare_dram_parameter("W2T", [128, 2304], DT_MM, isOutput=False)
    W3T = nc.declare_dram_parameter("W3T", [128, 576], DT_MM, isOutput=False)
    WeT2 = nc.declare_dram_parameter("WeT2", [64, 2304], DT_MM, isOutput=False)
    W3fT = nc.declare_dram_parameter("W3fT", [64, 7], DT_MM, isOutput=False)
    fc3b = nc.declare_dram_parameter("fc3b_rep", [64, 7], F32, isOutput=False)
    b2c = nc.declare_dram_parameter("b2c", [128, 1], F32, isOutput=False)
    b3c = nc.declare_dram_parameter("b3c", [64, 1], F32, isOutput=False)
    beffc = nc.declare_dram_parameter("beffc", [64, 1], F32, isOutput=False)
    Z = nc.declare_dram_parameter("Z", [1, 19200], DT_MM, isOutput=False)
    ONESR = nc.declare_dram_parameter("ONESR", [1, 19200], DT_MM, isOutput=False)
    IDENT = nc.declare_dram_parameter("IDENT", [36, 36], F32, isOutput=False)
    out = nc.declare_dram_parameter("out", [BPC, 7], F32, isOutput=True)
    dbg = {}
    if debug:
        for nm, shp in [("dbg_act1_0", [128, G * 676]), ("dbg_act1_1", [128, G * 676]),
                        ("dbg_act2", [128, BPC * 196]), ("dbg_out3", [64, BPC * 36]),
                        ("dbg_q", [64, 64]), ("dbg_attn", [64, 36]),
                        ("dbg_gT", [64, 64]), ("dbg_sc", [36, 64])]:
            dbg[nm] = nc.declare_dram_parameter(nm, shp, F32, isOutput=True)

    r = lambda ap: ap

    with tile.TileContext(nc) as tc, contextlib.ExitStack() as ctx:
        wp = ctx.enter_context(tc.tile_pool(name="weights", bufs=1))
        ap_pool = ctx.enter_context(tc.tile_pool(name="acts", bufs=1))
        cp = ctx.enter_context(tc.tile_pool(name="im2col", bufs=2))
        t1p = ctx.enter_context(tc.tile_pool(name="t1", bufs=3))
        e2p = ctx.enter_context(tc.tile_pool(name="ev2", bufs=6))
        e3p = ctx.enter_context(tc.tile_pool(name="ev3", bufs=3))

        # ---- group-0 input chain first, ahead of the bulk weight DMAs ----
        # xpq: zero-padded x staged as two x-parity planes per image:
        # cols p'*1250 + y_p*25 + x'_p with padded x_p = 2*x'_p + p'.
        # Host-deinterleaved x (see _prep_x) makes these loads contiguous.
        xpqs = [ap_pool.tile([8, 2502], DT_MM, tag=f"xpq{pp}",
                             name=f"xpq{pp}") for pp in range(2)]

        def load_xpq(g):
            nc.gpsimd.dma_start(out=xpqs[g % 2][:],
                                in_=x[G * g:G * (g + 1), :])

        def load_taps(g, imt, b0=0, b1=G):
            """im2col taps: row k, parity-q block <- padded parity plane.
            Output-x parity q tap (dy,dx) reads plane (q+dx)%2 shifted.
            gpsimd (SWDGE): ~4x faster than HWDGE for this descriptor
            pattern (measured)."""
            xpq = xpqs[g % 2]
            ivk = imt.rearrange("p (qq c) -> p qq c", qq=2)
            for q in range(2):
                eng = (nc.sync, nc.gpsimd)[q]
                for k in range(9):
                    dy, dx = divmod(k, 3)
                    lo = 1250 * ((q + dx) % 2) + 25 * dy + (q + dx) // 2
                    eng.dma_start(
                        out=ivk[k:k + 1, q, 1200 * b0:1200 * b1],
                        in_=xpq[b0:b1, lo:lo + 1200])

        # Persistent im2col tiles (group-parity double buffer), [128,
        # 2*G*1152]: rows 0-8 hold tap k (rewritten per group), row 9 the
        # ones-row contracting the folded bias, rows 10-64 zero. K=65
        # keeps conv1 matmuls in the full 128x128 PE tile mode: mixing
        # 32-row-tiled matmuls with conv2's full-mode ones halves the PE
        # clock around every switch (measured: 288-col MMs at 240ns).
        imts = [ap_pool.tile([128, G * 2400], DT_MM, tag=f"imt{pp}",
                             name=f"imt{pp}") for pp in range(2)]
        load_xpq(0)
        # early-critical zero region via DVE memset (no DMA traffic):
        # conv1 contracts imt rows 10-64 (all-zero weights there guard
        # against only-finite garbage, so they must be initialized).
        # Engines need base_partition 0, so clear 0:65 and let the taps
        # overwrite rows 0-9 afterwards.
        load_taps(0, imts[0][:])

        # ---- load weights (ahead of the bulk zero-fills: the first
        # matmuls need them; multi-MB fills would delay them ~40us) ----
        w1t = wp.tile([128, 256], DT_MM)
        nc.sync.dma_start(out=w1t[:], in_=W1T[:])
        w2t = wp.tile([128, 2304], DT_MM)
        nc.sync.dma_start(out=w2t[:], in_=W2T[:])
        w3t = wp.tile([128, 576], DT_MM)
        nc.sync.dma_start(out=w3t[:], in_=W3T[:])
        wet = wp.tile([64, 2304], DT_MM)
        nc.sync.dma_start(out=wet[:], in_=WeT2[:])
        w3f = wp.tile([64, 7], DT_MM)
        nc.sync.dma_start(out=w3f[:], in_=W3fT[:])
        fc3b_t = wp.tile([64, 7], F32)
        nc.sync.dma_start(out=fc3b_t[:], in_=fc3b[:])
        b2t = wp.tile([128, 1], F32)
        nc.sync.dma_start(out=b2t[:], in_=b2c[:])
        b3t = wp.tile([64, 1], F32)
        nc.sync.dma_start(out=b3t[:], in_=b3c[:])
        bet = wp.tile([64, 1], F32)
        nc.sync.dma_start(out=bet[:], in_=beffc[:])
        ident = wp.tile([36, 36], F32)
        nc.sync.dma_start(out=ident[:], in_=IDENT[:])
        ones1 = wp.tile([1, 64], DT_MM)
        nc.sync.dma_start(out=ones1[:], in_=ONESR[:, 0:64])


        # ---- persistent activation buffers ----
        act1 = [[ap_pool.tile([128, G * 676], DT_MM, tag=f"act1_{pp}_{h}",
                              name=f"act1_{pp}_{h}") for h in range(2)]
                for pp in range(2)]
        act2 = ap_pool.tile([128, BPC * 196], DT_MM)
        out3 = ap_pool.tile([64, BPC * 36], DT_MM)
        def fill_imt(pp):
            nc.sync.dma_start(out=imts[pp][9:10, :], in_=ONESR[:, :G * 2400])
            for c in range(4):
                nc.sync.dma_start(
                    out=imts[pp][10:65, 4800 * c:4800 * (c + 1)],
                    in_=Z[:, :4800].to_broadcast((55, 4800)))

        def fill_act1(pp):
            for h in range(2):
                for c in range(2):
                    nc.sync.dma_start(
                        out=act1[pp][h][:, 2704 * c:2704 * (c + 1)],
                        in_=Z[:, :2704].to_broadcast((128, 2704)))

        # one-time fills: early-needed ones via DVE memset, late-needed
        # ones as chunked sync DMAs so nothing gates the early pipeline
        for h in range(2):
            nc.vector.memset(act1[0][h][:], 0.0)
        fill_imt(0)
        fill_imt(1)
        fill_act1(1)
        for c in range(4):
            nc.sync.dma_start(
                out=act2[:, 3136 * c:3136 * (c + 1)],
                in_=Z[:, :3136].to_broadcast((128, 3136)))

        with contextlib.ExitStack() as cctx:
            ps1 = cctx.enter_context(tc.tile_pool(name="ps1", bufs=1, space="PSUM"))
            ps2 = cctx.enter_context(tc.tile_pool(name="ps2", bufs=2, space="PSUM"))

            # PE warm-up: ~2.5us of dummy matmuls as soon as the weights
            # land, so the HAM clock gate reaches 8/8 before real work
            # (cold MMs run at 1.2 GHz instead of 2.4).
            psW = ps1.tile([128, 1536], F32, tag="psE", name="psW")
            for i in range(24):
                nc.tensor.matmul(out=psW[:, 0:512], lhsT=r(w1t[0:65, 0:128]),
                                 rhs=r(w2t[0:65, 0:512]), start=True, stop=True)

            def conv1_h(g, ci, imt, h):
                """conv1 half h of one image: x-parity matmuls + pooled/
                relu'd write to act1."""
                iv = imt.rearrange("p (qq b y x) -> p qq b y x",
                                   qq=2, y=48, x=25)
                if True:
                    # psum layout: 3 banks x (16 y-rows x 24 x-cols = 384)
                    psE = ps1.tile([128, 1536], F32, tag="psE", name="psE")
                    psO = ps1.tile([128, 1536], F32, tag="psO", name="psO")
                    for bk in range(3):
                        for q, ps in ((0, psE), (1, psO)):
                            nc.tensor.matmul(
                                out=ps[:, 512 * bk:512 * bk + 384],
                                lhsT=r(w1t[0:65, 128 * h:128 * (h + 1)]),
                                rhs=r(iv[0:65, q, ci,
                                         16 * bk:16 * bk + 16, 0:24]),
                                start=True, stop=True)
                    psEv = psE[:].rearrange("p (k c) -> p k c", c=512)[:, :, 0:384]
                    psOv = psO[:].rearrange("p (k c) -> p k c", c=512)[:, :, 0:384]
                    # odd parity: relu-evac (bias already in psum via ones-row)
                    oddr = t1p.tile([128, 1152], DT_MM, tag="oddr", name="oddr")
                    oddv = oddr[:].rearrange("p (k c) -> p k c", c=384)
                    nc.scalar.activation(out=oddv, in_=psOv, func=RELU)
                    # max(even, relu(odd)) == relu(max(even, odd)) elementwise
                    m1 = t1p.tile([128, 1152], DT_MM, tag="m1", name="m1")
                    m1v3 = m1[:].rearrange("p (k c) -> p k c", c=384)
                    nc.vector.tensor_max(m1v3, psEv, oddv)
                    # y-pair max -> act1 padded interior (image ci)
                    m1v = m1[:].rearrange("p (y x) -> p y x", x=24)
                    dst = act1[g % 2][h][:].rearrange(
                        "p (b y x) -> p b y x", y=26, x=26)[:, ci, 1:25, 1:25]
                    eng = nc.gpsimd if GPS_YMAX else nc.vector
                    eng.tensor_max(dst, m1v[:, 0:48:2, :], m1v[:, 1:48:2, :])

            def conv2_rr(g, bb, rr):
                """conv2 for image bb of group g, output row-half rr."""
                a1v = [act1[g % 2][h][:].rearrange(
                    "p (b y x) -> p b y x", y=26, x=26) for h in range(2)]
                a2v = act2[:].rearrange("p (b y x) -> p b y x", y=14, x=14)
                if True:
                    ps = ps2.tile([128, 288], F32, tag="ps2")
                    i = 0
                    for t in range(9):
                        dy, dx = divmod(t, 3)
                        for h in range(2):
                            nc.tensor.matmul(
                                out=ps[:],
                                lhsT=r(w2t[:, (t * 2 + h) * 128:(t * 2 + h + 1) * 128]),
                                rhs=r(a1v[h][:, bb, 12 * rr + dy:12 * rr + dy + 12,
                                             dx:dx + 24]),
                                start=(i == 0), stop=(i == 17))
                            i += 1
                    psv = ps[:].rearrange("p (y x) -> p y x", x=24)
                    todd2 = e2p.tile([128, 144], DT_MM, tag="todd2")
                    todd2v = todd2[:].rearrange("p (y x) -> p y x", x=12)
                    nc.scalar.activation(out=todd2v, in_=psv[:, :, 1:24:2],
                                         func=RELU, bias=b2t[:])
                    t1c = e2p.tile([128, 144], DT_MM, tag="t1c2")
                    t1cv = t1c[:].rearrange("p (y x) -> p y x", x=12)
                    nc.vector.scalar_tensor_tensor(
                        out=t1cv, in0=psv[:, :, 0:24:2], scalar=b2t[:],
                        in1=todd2v, op0=ADD, op1=MAX)
                    dst = a2v[:, g * G + bb, 6 * rr + 1:6 * rr + 7, 1:13]
                    nc.vector.tensor_max(dst, t1cv[:, 0:12:2, :], t1cv[:, 1:12:2, :])

            # ---- group loop: conv1(g) interleaved with conv2(g-1) ----
            prev_g = None
            for g in range(NG):
                imt = imts[g % 2]
                if g != 0:
                    load_xpq(g)
                    load_taps(g, imt[:])
                for ci in range(G):
                    # fine interleave: each conv2 chain fills the PE while
                    # the preceding conv1 half's psum drains through the
                    # pool engines
                    for h in range(2):
                        if prev_g is not None:
                            conv2_rr(prev_g, ci, h)
                        conv1_h(g, ci, imt[:], h)
                prev_g = g
            for ci in range(G):
                for rr in range(2):
                    conv2_rr(prev_g, ci, rr)

        # ---- conv3 (all groups done; act2 complete) ----
        with contextlib.ExitStack() as cctx:
            ps3 = cctx.enter_context(tc.tile_pool(name="ps3", bufs=3, space="PSUM"))
            psq = cctx.enter_context(tc.tile_pool(name="psq", bufs=1, space="PSUM"))
            pssc = cctx.enter_context(tc.tile_pool(name="pssc", bufs=1, space="PSUM"))
            psT = cctx.enter_context(tc.tile_pool(name="psT", bufs=1, space="PSUM"))

            a2v = act2[:].rearrange("p (b y x) -> p b y x", y=14, x=14)
            o3v = out3[:].rearrange("p (b hw) -> p b hw", hw=36)
            for t in range(32):  # image pairs
                ps = ps3.tile([64, 288], F32, tag="ps3")
                for k in range(9):
                    dy, dx = divmod(k, 3)
                    nc.tensor.matmul(
                        out=ps[:],
                        lhsT=r(w3t[:, 64 * k:64 * (k + 1)]),
                        rhs=r(a2v[:, 2 * t:2 * t + 2, dy:dy + 12, dx:dx + 12]),
                        start=(k == 0), stop=(k == 8))
                psv = ps[:].rearrange("p (b y x) -> p b y x", y=12, x=12)
                todd3 = e3p.tile([64, 144], DT_MM, tag="todd3")
                todd3v = todd3[:].rearrange("p (b y x) -> p b y x", y=12, x=6)
                nc.scalar.activation(out=todd3v, in_=psv[:, :, :, 1:12:2],
                                     func=RELU, bias=b3t[:])
                t1c = e3p.tile([64, 144], DT_MM, tag="t1c3")
                t1cv = t1c[:].rearrange("p (b y x) -> p b y x", y=12, x=6)
                nc.vector.scalar_tensor_tensor(
                    out=t1cv, in0=psv[:, :, :, 0:12:2], scalar=b3t[:],
                    in1=todd3v, op0=ADD, op1=MAX)
                nc.vector.tensor_max(
                    o3v[:, 2 * t:2 * t + 2, :].rearrange("p b (y x) -> p b y x", x=6),
                    t1cv[:, :, 0:12:2, :], t1cv[:, :, 1:12:2, :])

            # ---- q = W_eff @ out4 + b_eff : accumulate over hw ----
            psq_t = psq.tile([64, 64], F32)
            for hw in range(36):
                nc.tensor.matmul(
                    out=psq_t[:],
                    lhsT=r(wet[:, 64 * hw:64 * (hw + 1)]),
                    rhs=r(out3[:, hw:2304:36]),
                    start=(hw == 0), stop=(hw == 35))
            q_sb = ap_pool.tile([64, 64], DT_MM)
            nc.vector.tensor_scalar_add(q_sb[:], psq_t[:], bet[:])

            # ---- scores: per-image matmuls -> [36, 64] psum ----
            pssc_t = pssc.tile([36, 64], F32)
            for b in range(BPC):
                nc.tensor.matmul(
                    out=pssc_t[:, b:b + 1],
                    lhsT=out3[:, 36 * b:36 * (b + 1)],
                    rhs=q_sb[:, b:b + 1],
                    start=True, stop=True)
            sc_sb = ap_pool.tile([36, 64], F32)
            nc.vector.tensor_copy(sc_sb[:], pssc_t[:])
            psT_t = psT.tile([64, 36], F32)
            nc.tensor.transpose(psT_t[:], sc_sb[:], ident[:])

            # ---- softmax over hw (free dim) ----
            nmx = ap_pool.tile([64, 1], F32)
            nc.vector.tensor_reduce(out=nmx[:], in_=psT_t[:],
                                    op=mybir.AluOpType.max,
                                    axis=mybir.AxisListType.X, negate=True)
            e_t = ap_pool.tile([64, 36], F32)
            nc.scalar.activation(out=e_t[:], in_=psT_t[:], func=EXP, bias=nmx[:])
            z = ap_pool.tile([64, 1], F32)
            nc.vector.tensor_reduce(out=z[:], in_=e_t[:],
                                    op=mybir.AluOpType.add,
                                    axis=mybir.AxisListType.X)
            rz = ap_pool.tile([64, 1], F32)
            nc.vector.reciprocal(rz[:], z[:])
            attn = ap_pool.tile([64, 36], DT_MM)
            nc.vector.tensor_scalar_mul(attn[:], e_t[:], rz[:])
            if debug:
                nc.gpsimd.dma_start(out=dbg["dbg_act1_0"][:], in_=act1[0][0][:])
                nc.gpsimd.dma_start(out=dbg["dbg_act1_1"][:], in_=act1[0][1][:])
                nc.gpsimd.dma_start(out=dbg["dbg_act2"][:], in_=act2[:])
                nc.gpsimd.dma_start(out=dbg["dbg_out3"][:], in_=out3[:])
                nc.gpsimd.dma_start(out=dbg["dbg_q"][:], in_=q_sb[:])
                nc.gpsimd.dma_start(out=dbg["dbg_attn"][:], in_=attn[:])
                nc.sync.dma_start(out=dbg["dbg_sc"][:], in_=sc_sb[:])

        # ---- g_mod + fc3 ----
        with contextlib.ExitStack() as cctx:
            psab = cctx.enter_context(tc.tile_pool(name="psab", bufs=1, space="PSUM"))
            psf = cctx.enter_context(tc.tile_pool(name="psf", bufs=1, space="PSUM"))

            attn_flat = ap_pool.tile([1, 2304], DT_MM)
            nc.sync.dma_start(out=attn_flat[:], in_=attn[:])
            psab_t = psab.tile([64, 2304], F32)
            for c in range(5):
                lo = 512 * c
                hi = min(lo + 512, 2304)
                nc.tensor.matmul(out=psab_t[:, lo:hi], lhsT=r(ones1[:]),
                                 rhs=r(attn_flat[:, lo:hi]), start=True, stop=True)
            # in-place: out3 is not needed after this product
            gT = ap_pool.tile([64, 64], DT_MM)
            o3r = out3[:].rearrange("p (b hw) -> p b hw", hw=36)
            with nc.allow_low_precision(reason="bf16 output of attn-weighted sum"):
                for half in range(2):
                    cols = slice(1152 * half, 1152 * (half + 1))
                    nc.vector.tensor_mul(out3[:, cols], out3[:, cols],
                                         psab_t[:, cols])
                    nc.vector.tensor_reduce(
                        out=gT[:, 32 * half:32 * (half + 1)],
                        in_=o3r[:, 32 * half:32 * (half + 1), :],
                        op=mybir.AluOpType.add, axis=mybir.AxisListType.X)

            if debug:
                nc.gpsimd.dma_start(out=dbg["dbg_gT"][:], in_=gT[:])
            psf_t = psf.tile([64, 7], F32)
            nc.tensor.matmul(out=psf_t[:], lhsT=gT[:],
                             rhs=w3f[:], start=True, stop=True)
            out_sb = ap_pool.tile([64, 7], F32)
            nc.vector.tensor_add(out_sb[:], psf_t[:], fc3b_t[:])
            nc.sync.dma_start(out=out[:], in_=out_sb[:])

    _split_excess_waits(nc)
    return nc


def kernel(**inputs):
    from concourse.bass_utils import run_bass_kernel_spmd

    w = _prep_weights({k: np.asarray(v, np.float32) for k, v in inputs.items()
                       if k != 'x'})
    npdt = mybir.dt.np(DT_MM)
    for k in ('W1T', 'W2T', 'W3T', 'WeT2', 'W3fT', 'Z', 'ONESR'):
        w[k] = w[k].astype(npdt)
    xs = _prep_x(inputs['x']).astype(npdt)

    nc = build_program()
    in_maps = []
    for c in range(N_CORES):
        m = {'x': np.ascontiguousarray(xs[BPC * c:BPC * (c + 1)])}
        m.update({k: v for k, v in w.items()})
        in_maps.append(m)
    res = run_bass_kernel_spmd(nc, in_maps, list(range(N_CORES)))
    outs = [res.results[c]['out'] for c in range(N_CORES)]
    return np.concatenate(outs, axis=0).astype(np.float32)


if __name__ == '__main__':
    rng = np.random.default_rng(0)
    fake = {
        'x': rng.standard_normal((512, 1, 48, 48), dtype=np.float32),
        'conv1_w': rng.standard_normal((256, 1, 3, 3), dtype=np.float32) * 0.05,
        'conv1_b': np.zeros(256, np.float32),
        'bn1_g': np.ones(256, np.float32), 'bn1_b': np.zeros(256, np.float32),
        'bn1_m': np.zeros(256, np.float32), 'bn1_v': np.ones(256, np.float32),
        'conv2_w': rng.standard_normal((128, 256, 3, 3), dtype=np.float32) * 0.05,
        'conv2_b': np.zeros(128, np.float32),
        'bn2_g': np.ones(128, np.float32), 'bn2_b': np.zeros(128, np.float32),
        'bn2_m': np.zeros(128, np.float32), 'bn2_v': np.ones(128, np.float32),
        'conv3_w': rng.standard_normal((64, 128, 3, 3), dtype=np.float32) * 0.05,
        'conv3_b': np.zeros(64, np.float32),
        'bn3_g': np.ones(64, np.float32), 'bn3_b': np.zeros(64, np.float32),
        'bn3_m': np.zeros(64, np.float32), 'bn3_v': np.ones(64, np.float32),
        'fc1_w': rng.standard_normal((512, 2304), dtype=np.float32) * 0.05,
        'fc1_b': np.zeros(512, np.float32),
        'fc2_w': rng.standard_normal((256, 512), dtype=np.float32) * 0.05,
        'fc2_b': np.zeros(256, np.float32),
        'att_w': rng.standard_normal((64, 256), dtype=np.float32) * 0.05,
        'att_b': np.zeros(64, np.float32),
        'fc3_w': rng.standard_normal((7, 64), dtype=np.float32) * 0.05,
        'fc3_b': np.zeros(7, np.float32),
    }
    print(kernel(**fake).shape)


# revision 34
# speedup vs baseline: 1.3614x; 1.0200x over previous
"""Trainium2 Bass kernel for nn_BaselineModel_80796924772520 (dense_cnn).

Self-contained: kernel(**inputs) -> np.ndarray [512, 7] float32.

Strategy: pure data parallelism over 8 NeuronCores (64 images each).
 - BN folded into conv weights/biases on host (eval-mode BN is affine).
 - fc1/fc2/att collapse into one linear map W_eff [64, 2304] on host
   (reference has no nonlinearity between them).
 - conv1 (C_in=1, K=9): x-parity decomposition on two PE row-quadrants.
   Quadrant q in {0,1} computes the even-x / odd-x conv outputs
   concurrently (32-row PE tiling), contracting K=10 rows: 9 im2col
   taps + a ones-row that adds the folded bias inside the matmul.
   Maxpool becomes max(even, odd) in x (one DVE op against the
   ACT-relu-evacuated odd parity) then a strided y-pair max; relu is
   folded into the max tree via max(a, relu(b)) == relu(max(a, b)).
 - conv2/conv3: 9-tap shifted-window accumulating matmuls over
   zero-padded SBUF activations; pooling uses an ACT relu+bias
   evacuation of the odd-x columns, a DVE scalar_tensor_tensor
   (even + bias) max odd, and a DVE y-pair max that writes the padded
   activation (or out3) directly - no separate bias/relu pass.
 - attention: per-image [64x36]^T@[64x1] matmuls -> PE transpose ->
   softmax -> broadcast-matmul with ones -> multiply+segmented reduce.
"""
import sys
if '/opt/trn_rl_repo' not in sys.path:
    sys.path.insert(0, '/opt/trn_rl_repo')

import contextlib
import numpy as np

import concourse.bass as bass
import concourse.mybir as mybir
import concourse.tile as tile

F32 = mybir.dt.float32
BF16 = mybir.dt.bfloat16
DT_MM = BF16
RELU = mybir.ActivationFunctionType.Relu
EXP = mybir.ActivationFunctionType.Exp
ADD = mybir.AluOpType.add
MAX = mybir.AluOpType.max

N_CORES = 8
B_TOTAL = 512
BPC = B_TOTAL // N_CORES   # 64 images per core
G = 8                      # images per group
NG = BPC // G              # 8 groups
EPS = 1e-5
GPS_YMAX = False           # offload conv1 y-max to gpsimd

_MAX_WAITS = 1  # this walrus build supports 1 sync-wait per instruction


def _install_tile_fixups():
    """The nix walrus here allows only ONE sync-wait per instruction; Tile's
    exit drain aggregates one wait per live proc onto a single Drain. Spread
    the waits across spare SP nops emitted just before the drain."""
    if getattr(tile.TileContext, '_drain_patched', False):
        return

    def _patched(self, tick_clock, wait_clock):
        from concourse.vector_clock import ScopedClock
        nc = self.nc
        nops = [nc.sync.nop().ins for _ in range(32)]
        drain_inst = nc.sync.drain()
        wait_clock.add_sem_waits(
            drain_inst.ins, ScopedClock({None: tick_clock.global_clock}))
        si = drain_inst.ins.sync_info
        if si is not None and len(si.on_wait) > _MAX_WAITS:
            waits = list(si.on_wait)
            drain_inst.ins.sync_info = mybir.SyncInfo(
                on_wait=waits[:_MAX_WAITS], on_update=list(si.on_update))
            rest = waits[_MAX_WAITS:]
            for i in range(0, len(rest), _MAX_WAITS):
                nops[i // _MAX_WAITS].sync_info = mybir.SyncInfo(
                    on_wait=rest[i:i + _MAX_WAITS], on_update=[])
        nc.all_engine_barrier()
        popped = nc._tile_sem_poison_stack.pop()
        assert popped is self._sem_poison
        nc.clear_and_free_semaphores(list(self.sems.allocated().values()))
        nc.all_engine_barrier()

    tile.TileContext._drain_and_barrier = _patched
    tile.TileContext._drain_patched = True


def _split_excess_waits(nc):
    """This walrus allows one sync-wait per instruction. Hoist excess waits
    onto same-engine nops inserted immediately before the instruction
    (sequential waits on one engine are equivalent to a combined wait)."""
    idx = 0
    for f in nc.m.functions:
        for b in f.blocks:
            out, changed = [], False
            for ins in b.instructions:
                si = ins.sync_info
                if si is not None and len(si.on_wait) > _MAX_WAITS:
                    waits = list(si.on_wait)
                    extra, keep = waits[:-_MAX_WAITS], waits[-_MAX_WAITS:]
                    for j in range(0, len(extra), _MAX_WAITS):
                        nop = mybir.InstNoOp(name=f"I-wsplit-{idx}")
                        idx += 1
                        nop.engine = ins.engine
                        nop.sync_info = mybir.SyncInfo(
                            on_wait=extra[j:j + _MAX_WAITS], on_update=[])
                        nc.register_instruction(nop, overwrite=True)
                        out.append(nop)
                    ins.sync_info = mybir.SyncInfo(
                        on_wait=keep, on_update=list(si.on_update))
                    changed = True
                out.append(ins)
            if changed:
                b.instructions = out


def _prep_weights(p):
    """Fold BN, collapse FC chain, lay out weights for the device program."""
    def fold(w, b, g, be, m, v):
        inv = (g / np.sqrt(v + EPS)).astype(np.float32)
        wf = (w * inv[:, None, None, None]).astype(np.float32)
        bf = ((b - m) * inv + be).astype(np.float32)
        return wf, bf

    w1, b1 = fold(p['conv1_w'], p['conv1_b'], p['bn1_g'], p['bn1_b'], p['bn1_m'], p['bn1_v'])
    w2, b2 = fold(p['conv2_w'], p['conv2_b'], p['bn2_g'], p['bn2_b'], p['bn2_m'], p['bn2_v'])
    w3, b3 = fold(p['conv3_w'], p['conv3_b'], p['bn3_g'], p['bn3_b'], p['bn3_m'], p['bn3_v'])

    # conv1 lhsT [128, 256]: rows 32q+k (k = 3*ky+kx) = w1[c, 0, ky, kx] for
    # q in {0,1} (even-x / odd-x PE quadrants); row 32q+9 = folded bias
    # (contracted against a ones-row in the im2col buffer).
    W1T = np.zeros((128, 256), np.float32)
    W1T[0:9, :] = w1.reshape(256, 9).T
    W1T[9, :] = b1
    # conv2 lhsT [128, 2304]: [p, t*256 + h*128 + m] = w2[m, 128h+p, t]
    W2T = np.ascontiguousarray(
        w2.reshape(128, 2, 128, 9).transpose(2, 3, 1, 0)  # [p, t, h, m]
    ).reshape(128, 2304)
    # conv3 lhsT [128, 576]: [p, t*64 + m] = w3[m, p, t]
    W3T = np.ascontiguousarray(
        w3.reshape(64, 128, 9).transpose(1, 2, 0)).reshape(128, 576)

    # FC chain collapse: q = out4 @ W_eff.T + b_eff
    fc1w, fc2w, attw = p['fc1_w'], p['fc2_w'], p['att_w']
    W_eff = (attw @ fc2w @ fc1w).astype(np.float32)          # [64, 2304]
    b_eff = (attw @ (fc2w @ p['fc1_b'] + p['fc2_b']) + p['att_b']).astype(np.float32)
    # WeT2 [64, 2304]: [c, hw*64 + m] = W_eff[m, c*36 + hw]
    WeT2 = np.ascontiguousarray(
        W_eff.reshape(64, 64, 36).transpose(1, 2, 0)).reshape(64, 2304)

    W3fT = np.ascontiguousarray(p['fc3_w'].T).astype(np.float32)  # [64, 7]
    fc3b_rep = np.broadcast_to(p['fc3_b'], (64, 7)).astype(np.float32).copy()

    b2c = b2.reshape(128, 1).astype(np.float32)
    b3c = b3.reshape(64, 1).astype(np.float32)
    beffc = b_eff.reshape(64, 1).astype(np.float32)

    return dict(W1T=W1T, W2T=W2T, W3T=W3T, WeT2=WeT2, W3fT=W3fT,
                fc3b_rep=fc3b_rep, b2c=b2c, b3c=b3c, beffc=beffc,
                Z=np.zeros((1, 19200), np.float32),
                ONESR=np.ones((1, 19200), np.float32),
                IDENT=np.eye(36, dtype=np.float32))


def _prep_x(x):
    """Zero-padded x deinterleaved into parity planes: plane p' holds
    padded columns x_p = 2*x'_p + p' as [50 y_p, 25 x'_p]; 2 pad cols at
    the end so shifted flat 1200-element im2col windows stay in-bounds.
    Host-side so every device DMA is fully contiguous."""
    xr = np.asarray(x, np.float32).reshape(-1, 48, 48)
    B = xr.shape[0]
    out = np.zeros((B, 2, 50, 25), np.float32)
    out[:, 0, 1:49, 1:25] = xr[:, :, 1::2]   # odd image cols -> even padded
    out[:, 1, 1:49, 0:24] = xr[:, :, 0::2]   # even image cols -> odd padded
    return np.concatenate([out.reshape(B, 2500),
                           np.zeros((B, 2), np.float32)], axis=1)


def build_program(debug=False):
    """Build the per-core SPMD Bass program. Returns nc."""
    _install_tile_fixups()
    nc = bass.Bass("TRN2", target_bir_lowering=False, debug=False)

    x = nc.declare_dram_parameter("x", [BPC, 2502], DT_MM, isOutput=False)
    W1T = nc.declare_dram_parameter("W1T", [128, 256], DT_MM, isOutput=False)
    W2T = nc.declare_dram_parameter("W2T", [128, 2304], DT_MM, isOutput=False)
    W3T = nc.declare_dram_parameter("W3T", [128, 576], DT_MM, isOutput=False)
    WeT2 = nc.declare_dram_parameter("WeT2", [64, 2304], DT_MM, isOutput=False)
    W3fT = nc.declare_dram_parameter("W3fT", [64, 7], DT_MM, isOutput=False)
    fc3b = nc.declare_dram_parameter("fc3b_rep", [64, 7], F32, isOutput=False)
    b2c = nc.declare_dram_parameter("b2c", [128, 1], F32, isOutput=False)
    b3c = nc.declare_dram_parameter("b3c", [64, 1], F32, isOutput=False)
    beffc = nc.declare_dram_parameter("beffc", [64, 1], F32, isOutput=False)
    Z = nc.declare_dram_parameter("Z", [1, 19200], DT_MM, isOutput=False)
    ONESR = nc.declare_dram_parameter("ONESR", [1, 19200], DT_MM, isOutput=False)
    IDENT = nc.declare_dram_parameter("IDENT", [36, 36], F32, isOutput=False)
    out = nc.declare_dram_parameter("out", [BPC, 7], F32, isOutput=True)
    dbg = {}
    if debug:
        for nm, shp in [("dbg_act1_0", [128, G * 676]), ("dbg_act1_1", [128, G * 676]),
                        ("dbg_act2", [128, BPC * 196]), ("dbg_out3", [64, BPC * 36]),
                        ("dbg_q", [64, 64]), ("dbg_attn", [64, 36]),
                        ("dbg_gT", [64, 64]), ("dbg_sc", [36, 64])]:
            dbg[nm] = nc.declare_dram_parameter(nm, shp, F32, isOutput=True)

    r = lambda ap: ap

    with tile.TileContext(nc) as tc, contextlib.ExitStack() as ctx:
        wp = ctx.enter_context(tc.tile_pool(name="weights", bufs=1))
        ap_pool = ctx.enter_context(tc.tile_pool(name="acts", bufs=1))
        cp = ctx.enter_context(tc.tile_pool(name="im2col", bufs=2))
        t1p = ctx.enter_context(tc.tile_pool(name="t1", bufs=3))
        e2p = ctx.enter_context(tc.tile_pool(name="ev2", bufs=6))
        e3p = ctx.enter_context(tc.tile_pool(name="ev3", bufs=3))

        # ---- group-0 input chain first, ahead of the bulk weight DMAs ----
        # xpq: zero-padded x staged as two x-parity planes per image:
        # cols p'*1250 + y_p*25 + x'_p with padded x_p = 2*x'_p + p'.
        # Host-deinterleaved x (see _prep_x) makes these loads contiguous.
        xpqs = [ap_pool.tile([8, 2502], DT_MM, tag=f"xpq{pp}",
                             name=f"xpq{pp}") for pp in range(2)]

        def load_xpq(g):
            nc.gpsimd.dma_start(out=xpqs[g % 2][:],
                                in_=x[G * g:G * (g + 1), :])

        def load_taps(g, imt, b0=0, b1=G):
            """im2col taps: row k, parity-q block <- padded parity plane.
            Output-x parity q tap (dy,dx) reads plane (q+dx)%2 shifted.
            gpsimd (SWDGE): ~4x faster than HWDGE for this descriptor
            pattern (measured)."""
            xpq = xpqs[g % 2]
            ivk = imt.rearrange("p (qq c) -> p qq c", qq=2)
            for q in range(2):
                eng = (nc.sync, nc.gpsimd)[q]
                for k in range(9):
                    dy, dx = divmod(k, 3)
                    lo = 1250 * ((q + dx) % 2) + 25 * dy + (q + dx) // 2
                    eng.dma_start(
                        out=ivk[k:k + 1, q, 1200 * b0:1200 * b1],
                        in_=xpq[b0:b1, lo:lo + 1200])

        # Persistent im2col tiles (group-parity double buffer), [128,
        # 2*G*1152]: rows 0-8 hold tap k (rewritten per group), row 9 the
        # ones-row contracting the folded bias, rows 10-64 zero. K=65
        # keeps conv1 matmuls in the full 128x128 PE tile mode: mixing
        # 32-row-tiled matmuls with conv2's full-mode ones halves the PE
        # clock around every switch (measured: 288-col MMs at 240ns).
        imts = [ap_pool.tile([128, G * 2400], DT_MM, tag=f"imt{pp}",
                             name=f"imt{pp}") for pp in range(2)]
        load_xpq(0)
        # early-critical zero region via DVE memset (no DMA traffic):
        # conv1 contracts imt rows 10-64 (all-zero weights there guard
        # against only-finite garbage, so they must be initialized).
        # Engines need base_partition 0, so clear 0:65 and let the taps
        # overwrite rows 0-9 afterwards.
        load_taps(0, imts[0][:])

        # ---- load weights (ahead of the bulk zero-fills: the first
        # matmuls need them; multi-MB fills would delay them ~40us) ----
        w1t = wp.tile([128, 256], DT_MM)
        nc.sync.dma_start(out=w1t[:], in_=W1T[:])
        w2t = wp.tile([128, 2304], DT_MM)
        nc.sync.dma_start(out=w2t[:], in_=W2T[:])
        w3t = wp.tile([128, 576], DT_MM)
        nc.sync.dma_start(out=w3t[:], in_=W3T[:])
        wet = wp.tile([64, 2304], DT_MM)
        nc.sync.dma_start(out=wet[:], in_=WeT2[:])
        w3f = wp.tile([64, 7], DT_MM)
        nc.sync.dma_start(out=w3f[:], in_=W3fT[:])
        fc3b_t = wp.tile([64, 7], F32)
        nc.sync.dma_start(out=fc3b_t[:], in_=fc3b[:])
        b2t = wp.tile([128, 1], F32)
        nc.sync.dma_start(out=b2t[:], in_=b2c[:])
        b3t = wp.tile([64, 1], F32)
        nc.sync.dma_start(out=b3t[:], in_=b3c[:])
        bet = wp.tile([64, 1], F32)
        nc.sync.dma_start(out=bet[:], in_=beffc[:])
        ident = wp.tile([36, 36], F32)
        nc.sync.dma_start(out=ident[:], in_=IDENT[:])
        ones1 = wp.tile([1, 64], DT_MM)
        nc.sync.dma_start(out=ones1[:], in_=ONESR[:, 0:64])


        # ---- persistent activation buffers ----
        act1 = [[ap_pool.tile([128, G * 676], DT_MM, tag=f"act1_{pp}_{h}",
                              name=f"act1_{pp}_{h}") for h in range(2)]
                for pp in range(2)]
        act2 = ap_pool.tile([128, BPC * 196], DT_MM)
        out3 = ap_pool.tile([64, BPC * 36], DT_MM)
        def fill_imt(pp):
            nc.sync.dma_start(out=imts[pp][9:10, :], in_=ONESR[:, :G * 2400])
            for c in range(4):
                nc.sync.dma_start(
                    out=imts[pp][10:65, 4800 * c:4800 * (c + 1)],
                    in_=Z[:, :4800].to_broadcast((55, 4800)))

        def fill_act1(pp):
            for h in range(2):
                for c in range(2):
                    nc.sync.dma_start(
                        out=act1[pp][h][:, 2704 * c:2704 * (c + 1)],
                        in_=Z[:, :2704].to_broadcast((128, 2704)))

        # one-time fills: early-needed ones via DVE memset, late-needed
        # ones as chunked sync DMAs so nothing gates the early pipeline
        for h in range(2):
            nc.vector.memset(act1[0][h][:], 0.0)
        fill_imt(0)
        fill_imt(1)
        fill_act1(1)
        for c in range(4):
            nc.sync.dma_start(
                out=act2[:, 3136 * c:3136 * (c + 1)],
                in_=Z[:, :3136].to_broadcast((128, 3136)))

        with contextlib.ExitStack() as cctx:
            ps1 = cctx.enter_context(tc.tile_pool(name="ps1", bufs=1, space="PSUM"))
            ps2 = cctx.enter_context(tc.tile_pool(name="ps2", bufs=2, space="PSUM"))

            # PE warm-up: ~2.5us of dummy matmuls as soon as the weights
            # land, so the HAM clock gate reaches 8/8 before real work
            # (cold MMs run at 1.2 GHz instead of 2.4).
            psW = ps1.tile([128, 1536], F32, tag="psE", name="psW")
            for i in range(24):
                nc.tensor.matmul(out=psW[:, 0:512], lhsT=r(w1t[0:65, 0:128]),
                                 rhs=r(w2t[0:65, 0:512]), start=True, stop=True)

            def conv1_h(g, ci, imt, h):
                """conv1 half h of one image: x-parity matmuls + pooled/
                relu'd write to act1."""
                iv = imt.rearrange("p (qq b y x) -> p qq b y x",
                                   qq=2, y=48, x=25)
                if True:
                    # psum layout: 3 banks x (16 y-rows x 24 x-cols = 384)
                    psE = ps1.tile([128, 1536], F32, tag="psE", name="psE")
                    psO = ps1.tile([128, 1536], F32, tag="psO", name="psO")
                    for q, ps in ((0, psE), (1, psO)):
                        for bk in range(3):
                            nc.tensor.matmul(
                                out=ps[:, 512 * bk:512 * bk + 384],
                                lhsT=r(w1t[0:65, 128 * h:128 * (h + 1)]),
                                rhs=r(iv[0:65, q, ci,
                                         16 * bk:16 * bk + 16, 0:24]),
                                start=True, stop=True)
                    psEv = psE[:].rearrange("p (k c) -> p k c", c=512)[:, :, 0:384]
                    psOv = psO[:].rearrange("p (k c) -> p k c", c=512)[:, :, 0:384]
                    # even parity: relu-evac (bias already in psum via the
                    # ones-row). Evacuating psE (written first, above) frees
                    # it earliest for the next half's first matmuls.
                    oddr = t1p.tile([128, 1152], DT_MM, tag="oddr", name="oddr")
                    oddv = oddr[:].rearrange("p (k c) -> p k c", c=384)
                    nc.scalar.activation(out=oddv, in_=psEv, func=RELU)
                    # max(odd, relu(even)) == relu(max(even, odd)) elementwise
                    m1 = t1p.tile([128, 1152], DT_MM, tag="m1", name="m1")
                    m1v3 = m1[:].rearrange("p (k c) -> p k c", c=384)
                    nc.vector.tensor_max(m1v3, psOv, oddv)
                    # y-pair max -> act1 padded interior (image ci)
                    m1v = m1[:].rearrange("p (y x) -> p y x", x=24)
                    dst = act1[g % 2][h][:].rearrange(
                        "p (b y x) -> p b y x", y=26, x=26)[:, ci, 1:25, 1:25]
                    eng = nc.gpsimd if GPS_YMAX else nc.vector
                    eng.tensor_max(dst, m1v[:, 0:48:2, :], m1v[:, 1:48:2, :])

            def conv2_rr(g, bb, rr):
                """conv2 for image bb of group g, output row-half rr."""
                a1v = [act1[g % 2][h][:].rearrange(
                    "p (b y x) -> p b y x", y=26, x=26) for h in range(2)]
                a2v = act2[:].rearrange("p (b y x) -> p b y x", y=14, x=14)
                if True:
                    ps = ps2.tile([128, 288], F32, tag="ps2")
                    i = 0
                    for t in range(9):
                        dy, dx = divmod(t, 3)
                        for h in range(2):
                            nc.tensor.matmul(
                                out=ps[:],
                                lhsT=r(w2t[:, (t * 2 + h) * 128:(t * 2 + h + 1) * 128]),
                                rhs=r(a1v[h][:, bb, 12 * rr + dy:12 * rr + dy + 12,
                                             dx:dx + 24]),
                                start=(i == 0), stop=(i == 17))
                            i += 1
                    psv = ps[:].rearrange("p (y x) -> p y x", x=24)
                    todd2 = e2p.tile([128, 144], DT_MM, tag="todd2")
                    todd2v = todd2[:].rearrange("p (y x) -> p y x", x=12)
                    nc.scalar.activation(out=todd2v, in_=psv[:, :, 1:24:2],
                                         func=RELU, bias=b2t[:])
                    t1c = e2p.tile([128, 144], DT_MM, tag="t1c2")
                    t1cv = t1c[:].rearrange("p (y x) -> p y x", x=12)
                    nc.vector.scalar_tensor_tensor(
                        out=t1cv, in0=psv[:, :, 0:24:2], scalar=b2t[:],
                        in1=todd2v, op0=ADD, op1=MAX)
                    dst = a2v[:, g * G + bb, 6 * rr + 1:6 * rr + 7, 1:13]
                    nc.vector.tensor_max(dst, t1cv[:, 0:12:2, :], t1cv[:, 1:12:2, :])

            # ---- group loop: conv1(g) interleaved with conv2(g-1) ----
            prev_g = None
            for g in range(NG):
                imt = imts[g % 2]
                if g != 0:
                    load_xpq(g)
                    load_taps(g, imt[:])
                for ci in range(G):
                    # fine interleave: each conv2 chain fills the PE while
                    # the preceding conv1 half's psum drains through the
                    # pool engines
                    for h in range(2):
                        if prev_g is not None:
                            conv2_rr(prev_g, ci, h)
                        conv1_h(g, ci, imt[:], h)
                prev_g = g
            for ci in range(G):
                for rr in range(2):
                    conv2_rr(prev_g, ci, rr)

        # ---- conv3 (all groups done; act2 complete) ----
        with contextlib.ExitStack() as cctx:
            ps3 = cctx.enter_context(tc.tile_pool(name="ps3", bufs=3, space="PSUM"))
            psq = cctx.enter_context(tc.tile_pool(name="psq", bufs=1, space="PSUM"))
            pssc = cctx.enter_context(tc.tile_pool(name="pssc", bufs=1, space="PSUM"))
            psT = cctx.enter_context(tc.tile_pool(name="psT", bufs=1, space="PSUM"))

            a2v = act2[:].rearrange("p (b y x) -> p b y x", y=14, x=14)
            o3v = out3[:].rearrange("p (b hw) -> p b hw", hw=36)
            for t in range(32):  # image pairs
                ps = ps3.tile([64, 288], F32, tag="ps3")
                for k in range(9):
                    dy, dx = divmod(k, 3)
                    nc.tensor.matmul(
                        out=ps[:],
                        lhsT=r(w3t[:, 64 * k:64 * (k + 1)]),
                        rhs=r(a2v[:, 2 * t:2 * t + 2, dy:dy + 12, dx:dx + 12]),
                        start=(k == 0), stop=(k == 8))
                psv = ps[:].rearrange("p (b y x) -> p b y x", y=12, x=12)
                todd3 = e3p.tile([64, 144], DT_MM, tag="todd3")
                todd3v = todd3[:].rearrange("p (b y x) -> p b y x", y=12, x=6)
                nc.scalar.activation(out=todd3v, in_=psv[:, :, :, 1:12:2],
                                     func=RELU, bias=b3t[:])
                t1c = e3p.tile([64, 144], DT_MM, tag="t1c3")
                t1cv = t1c[:].rearrange("p (b y x) -> p b y x", y=12, x=6)
                nc.vector.scalar_tensor_tensor(
                    out=t1cv, in0=psv[:, :, :, 0:12:2], scalar=b3t[:],
                    in1=todd3v, op0=ADD, op1=MAX)
                nc.vector.tensor_max(
                    o3v[:, 2 * t:2 * t + 2, :].rearrange("p b (y x) -> p b y x", x=6),
                    t1cv[:, :, 0:12:2, :], t1cv[:, :, 1:12:2, :])

            # ---- q = W_eff @ out4 + b_eff : accumulate over hw ----
            psq_t = psq.tile([64, 64], F32)
            for hw in range(36):
                nc.tensor.matmul(
                    out=psq_t[:],
                    lhsT=r(wet[:, 64 * hw:64 * (hw + 1)]),
                    rhs=r(out3[:, hw:2304:36]),
                    start=(hw == 0), stop=(hw == 35))
            q_sb = ap_pool.tile([64, 64], DT_MM)
            nc.vector.tensor_scalar_add(q_sb[:], psq_t[:], bet[:])

            # ---- scores: per-image matmuls -> [36, 64] psum ----
            pssc_t = pssc.tile([36, 64], F32)
            for b in range(BPC):
                nc.tensor.matmul(
                    out=pssc_t[:, b:b + 1],
                    lhsT=out3[:, 36 * b:36 * (b + 1)],
                    rhs=q_sb[:, b:b + 1],
                    start=True, stop=True)
            sc_sb = ap_pool.tile([36, 64], F32)
            nc.vector.tensor_copy(sc_sb[:], pssc_t[:])
            psT_t = psT.tile([64, 36], F32)
            nc.tensor.transpose(psT_t[:], sc_sb[:], ident[:])

            # ---- softmax over hw (free dim) ----
            nmx = ap_pool.tile([64, 1], F32)
            nc.vector.tensor_reduce(out=nmx[:], in_=psT_t[:],
                                    op=mybir.AluOpType.max,
                                    axis=mybir.AxisListType.X, negate=True)
            e_t = ap_pool.tile([64, 36], F32)
            nc.scalar.activation(out=e_t[:], in_=psT_t[:], func=EXP, bias=nmx[:])
            z = ap_pool.tile([64, 1], F32)
            nc.vector.tensor_reduce(out=z[:], in_=e_t[:],
                                    op=mybir.AluOpType.add,
                                    axis=mybir.AxisListType.X)
            rz = ap_pool.tile([64, 1], F32)
            nc.vector.reciprocal(rz[:], z[:])
            attn = ap_pool.tile([64, 36], DT_MM)
            nc.vector.tensor_scalar_mul(attn[:], e_t[:], rz[:])
            if debug:
                nc.gpsimd.dma_start(out=dbg["dbg_act1_0"][:], in_=act1[0][0][:])
                nc.gpsimd.dma_start(out=dbg["dbg_act1_1"][:], in_=act1[0][1][:])
                nc.gpsimd.dma_start(out=dbg["dbg_act2"][:], in_=act2[:])
                nc.gpsimd.dma_start(out=dbg["dbg_out3"][:], in_=out3[:])
                nc.gpsimd.dma_start(out=dbg["dbg_q"][:], in_=q_sb[:])
                nc.gpsimd.dma_start(out=dbg["dbg_attn"][:], in_=attn[:])
                nc.sync.dma_start(out=dbg["dbg_sc"][:], in_=sc_sb[:])

        # ---- g_mod + fc3 ----
        with contextlib.ExitStack() as cctx:
            psab = cctx.enter_context(tc.tile_pool(name="psab", bufs=1, space="PSUM"))
            psf = cctx.enter_context(tc.tile_pool(name="psf", bufs=1, space="PSUM"))

            attn_flat = ap_pool.tile([1, 2304], DT_MM)
            nc.sync.dma_start(out=attn_flat[:], in_=attn[:])
            psab_t = psab.tile([64, 2304], F32)
            for c in range(5):
                lo = 512 * c
                hi = min(lo + 512, 2304)
                nc.tensor.matmul(out=psab_t[:, lo:hi], lhsT=r(ones1[:]),
                                 rhs=r(attn_flat[:, lo:hi]), start=True, stop=True)
            # in-place: out3 is not needed after this product
            gT = ap_pool.tile([64, 64], DT_MM)
            o3r = out3[:].rearrange("p (b hw) -> p b hw", hw=36)
            with nc.allow_low_precision(reason="bf16 output of attn-weighted sum"):
                for half in range(2):
                    cols = slice(1152 * half, 1152 * (half + 1))
                    nc.vector.tensor_mul(out3[:, cols], out3[:, cols],
                                         psab_t[:, cols])
                    nc.vector.tensor_reduce(
                        out=gT[:, 32 * half:32 * (half + 1)],
                        in_=o3r[:, 32 * half:32 * (half + 1), :],
                        op=mybir.AluOpType.add, axis=mybir.AxisListType.X)

            if debug:
                nc.gpsimd.dma_start(out=dbg["dbg_gT"][:], in_=gT[:])
            psf_t = psf.tile([64, 7], F32)
            nc.tensor.matmul(out=psf_t[:], lhsT=gT[:],
                             rhs=w3f[:], start=True, stop=True)
            out_sb = ap_pool.tile([64, 7], F32)
            nc.vector.tensor_add(out_sb[:], psf_t[:], fc3b_t[:])
            nc.sync.dma_start(out=out[:], in_=out_sb[:])

    _split_excess_waits(nc)
    return nc


def kernel(**inputs):
    from concourse.bass_utils import run_bass_kernel_spmd

    w = _prep_weights({k: np.asarray(v, np.float32) for k, v in inputs.items()
                       if k != 'x'})
    npdt = mybir.dt.np(DT_MM)
    for k in ('W1T', 'W2T', 'W3T', 'WeT2', 'W3fT', 'Z', 'ONESR'):
        w[k] = w[k].astype(npdt)
    xs = _prep_x(inputs['x']).astype(npdt)

    nc = build_program()
    in_maps = []
    for c in range(N_CORES):
        m = {'x': np.ascontiguousarray(xs[BPC * c:BPC * (c + 1)])}
        m.update({k: v for k, v in w.items()})
        in_maps.append(m)
    res = run_bass_kernel_spmd(nc, in_maps, list(range(N_CORES)))
    outs = [res.results[c]['out'] for c in range(N_CORES)]
    return np.concatenate(outs, axis=0).astype(np.float32)


if __name__ == '__main__':
    rng = np.random.default_rng(0)
    fake = {
        'x': rng.standard_normal((512, 1, 48, 48), dtype=np.float32),
        'conv1_w': rng.standard_normal((256, 1, 3, 3), dtype=np.float32) * 0.05,
        'conv1_b': np.zeros(256, np.float32),
        'bn1_g': np.ones(256, np.float32), 'bn1_b': np.zeros(256, np.float32),
        'bn1_m': np.zeros(256, np.float32), 'bn1_v': np.ones(256, np.float32),
        'conv2_w': rng.standard_normal((128, 256, 3, 3), dtype=np.float32) * 0.05,
        'conv2_b': np.zeros(128, np.float32),
        'bn2_g': np.ones(128, np.float32), 'bn2_b': np.zeros(128, np.float32),
        'bn2_m': np.zeros(128, np.float32), 'bn2_v': np.ones(128, np.float32),
        'conv3_w': rng.standard_normal((64, 128, 3, 3), dtype=np.float32) * 0.05,
        'conv3_b': np.zeros(64, np.float32),
        'bn3_g': np.ones(64, np.float32), 'bn3_b': np.zeros(64, np.float32),
        'bn3_m': np.zeros(64, np.float32), 'bn3_v': np.ones(64, np.float32),
        'fc1_w': rng.standard_normal((512, 2304), dtype=np.float32) * 0.05,
        'fc1_b': np.zeros(512, np.float32),
        'fc2_w': rng.standard_normal((256, 512), dtype=np.float32) * 0.05,
        'fc2_b': np.zeros(256, np.float32),
        'att_w': rng.standard_normal((64, 256), dtype=np.float32) * 0.05,
        'att_b': np.zeros(64, np.float32),
        'fc3_w': rng.standard_normal((7, 64), dtype=np.float32) * 0.05,
        'fc3_b': np.zeros(7, np.float32),
    }
    print(kernel(**fake).shape)
